# revision 29
# baseline (speedup 1.0000x reference)
"""Trainium2 Bass kernel for nn_DrugResponsePrior (embedding_lookup).

Spec guarantees: cell_map < 100, is_missing in {0,1}, drug_map < 256.  So each
row depends only on cs = cell_map[idx]+100*is_missing[idx] (200 states) and
dm = drug_map[tidx] (256 drugs).

Fully data-parallel (8 cores x 8192 samples, no collectives), fp8 pipeline:
  1. cs/dm u8 tables in a 16-slab SBUF layout; per-sample lookup with gpsimd
     indirect_copy + one-hot slab mask + group-reduce matmul (bf16).
  2. Tables A = l2n(cell table) @ Wf1c + bf1 (200x200) and Bd = l2n(drug_emb)
     @ Wf1d (256x200) built on device, stored as scaled fp8 DoubleRow lhsT.
  3. Per 512-sample chunk: one-hot matrices (fp8) over cs/dm, h1/h2/fwd as
     fp8 DoubleRow matmuls (0.5 cyc/row), biases folded into pad rows or
     activation bias, softplus residual + cumsum via one bf16 9x9 matmul plus
     an exact f32 ln2-ramp add.
Output mu is written [9, BS] per core; host transposes/concats.
Numerics validated on host: rel_fro ~ 5e-4 (gate 2e-2) with these scales.
"""
import sys

if "/opt/trn_rl_repo" not in sys.path:
    sys.path.insert(0, "/opt/trn_rl_repo")

import numpy as np
import ml_dtypes

import concourse.bass as bass
import concourse.mybir as mybir
import concourse.tile as tile
from concourse.bass_utils import run_bass_kernel_spmd

f32 = mybir.dt.float32
bf16 = mybir.dt.bfloat16
fp8 = mybir.dt.float8e4
u16 = mybir.dt.uint16
u8 = mybir.dt.uint8

NP_BF16 = ml_dtypes.bfloat16
NP_FP8 = ml_dtypes.float8_e4m3

B = 65536
R = 262144
NDRUG = 256
NFEAT = 1024
CEMB = 1024
DEMB = 128
HID = 200
NDOSES = 9
NCORES = 8

BS = B // NCORES            # 8192 samples per core
P = 128
NG = 8                      # index groups (16 partitions each)
GS = BS // NG               # 1024 samples per group
SLAB = R // 16              # 16384 entries per slab partition
NCHUNK = BS // 512          # 16 chunks of 512 samples
EPS = 1e-12
LN2 = float(np.log(2.0))

# power-of-two scales for fp8 operands (validated on host: rel_fro ~ 5e-4)
S_CF = 8.0                  # cell_features
S_W1 = 64.0                 # W1
SP100 = S_CF * S_W1         # psum scale of cf @ W1
S1 = 256.0                  # A / Bd tables (h1 psum scale)
S_H = 64.0                  # h1 activations
S_W2 = 256.0                # Wf2
S_H2 = 64.0                 # h2 activations
S_W3 = 1024.0               # Wf3

_NC_CACHE = {}


def _split_sync_waits(nc, limit=1):
    """walrus accepts at most one sync-wait per instruction; hoist excess
    waits onto same-engine NoOps inserted just before."""
    ctr = 0
    for bb in nc.main_func.blocks:
        new_list = []
        for inst in bb.instructions:
            si = inst.sync_info
            if si is not None and si.on_wait and len(si.on_wait) > limit:
                waits = list(si.on_wait)
                head, tail = waits[:-limit], waits[-limit:]
                for j in range(0, len(head), limit):
                    nop = mybir.InstNoOp(name=f"waitnop-{ctr}", engine=inst.engine)
                    ctr += 1
                    nop.sync_info = mybir.SyncInfo(
                        on_wait=list(head[j : j + limit]), on_update=[]
                    )
                    new_list.append(nop)
                inst.sync_info = mybir.SyncInfo(
                    on_wait=list(tail),
                    on_update=list(si.on_update) if si.on_update else [],
                )
            new_list.append(inst)
        bb.instructions[:] = new_list
    return nc


def build_nc(split_waits=True):
    nc = bass.Bass(num_devices=NCORES)
    AF = mybir.ActivationFunctionType
    ALU = mybir.AluOpType
    DR = mybir.MatmulPerfMode.DoubleRow

    # ---------------- kernel I/O ----------------
    cs8 = nc.dram_tensor("cs8", [R], u8, kind="ExternalInput")
    dm8 = nc.dram_tensor("dm8", [R], u8, kind="ExternalInput")
    u_idx = nc.dram_tensor("u_idx", [P, GS // 16], u16, kind="ExternalInput")
    u_tidx = nc.dram_tensor("u_tidx", [P, GS // 16], u16, kind="ExternalInput")
    qrow_cs = nc.dram_tensor("qrow_cs", [NG * GS], bf16, kind="ExternalInput")
    qrow_dm = nc.dram_tensor("qrow_dm", [NG * GS], bf16, kind="ExternalInput")
    w18 = nc.dram_tensor("w18", [NFEAT, CEMB], fp8, kind="ExternalInput")
    cf8 = nc.dram_tensor("cf8", [NFEAT, 100], fp8, kind="ExternalInput")
    b1S = nc.dram_tensor("b1S", [CEMB], bf16, kind="ExternalInput")
    wf1c = nc.dram_tensor("wf1c", [CEMB, HID], bf16, kind="ExternalInput")
    bf1S1 = nc.dram_tensor("bf1S1", [HID], bf16, kind="ExternalInput")
    me_in = nc.dram_tensor("me_in", [100, CEMB], f32, kind="ExternalInput")
    de_bf = nc.dram_tensor("de_bf", [NDRUG, DEMB], bf16, kind="ExternalInput")
    deT_bf = nc.dram_tensor("deT_bf", [DEMB, NDRUG], bf16, kind="ExternalInput")
    wf1d = nc.dram_tensor("wf1d", [DEMB, HID], bf16, kind="ExternalInput")
    wf28 = nc.dram_tensor("wf28", [P, 2 * 208], fp8, kind="ExternalInput")
    wf38 = nc.dram_tensor("wf38", [P, 2 * 16], fp8, kind="ExternalInput")
    mu_s = nc.dram_tensor("mu_s", [NDOSES, BS], f32, kind="ExternalOutput")

    # internal DRAM rows for the per-sample cs/dm values (broadcast reload)
    cs_rowd = nc.dram_tensor("cs_rowd", [BS], bf16)
    dm_rowd = nc.dram_tensor("dm_rowd", [BS], bf16)

    # inline constants
    icol_c = nc.inline_tensor(
        np.arange(P, dtype=np.float32).reshape(P, 1), name="icol_c")
    qcol_c = nc.inline_tensor(
        (np.arange(P, dtype=np.float32) % 16).reshape(P, 1), name="qcol_c")
    grp_rd_c = nc.inline_tensor(  # [128, 8] group-reduce lhsT (bf16)
        np.array([[1.0 if (k // 16) == g else 0.0 for g in range(NG)]
                  for k in range(P)], NP_BF16), name="grp_rd_c")
    ones100_c = nc.inline_tensor(np.ones((1, 100), NP_BF16), name="ones100_c")
    ones19_c = nc.inline_tensor(np.ones((1, NDOSES), NP_BF16), name="ones19_c")
    # L8[k, o] = 1 iff dose k+1 contributes to output o (k+1 <= o)
    L8np = np.zeros((NDOSES - 1, NDOSES), NP_BF16)
    for k in range(NDOSES - 1):
        L8np[k, k + 1:] = 1.0
    L8_c = nc.inline_tensor(L8np, name="L8_c")
    ramp_c = nc.inline_tensor(
        (LN2 * np.arange(NDOSES, dtype=np.float32)).reshape(NDOSES, 1),
        name="ramp_c")
    ident_c = nc.inline_tensor(np.eye(100, dtype=np.float32), name="ident_c")


    with tile.TileContext(nc) as tc, \
            tc.tile_pool(name="sb", bufs=1) as sb, \
            tc.tile_pool(name="ps", bufs=1, space="PSUM") as ps:

        # ======== psum banks: pair-fused 2-bank tiles (8 banks total) ========
        h1m0p = ps.tile([P, 1024], f32, tag="h1m0p")    # 2 banks
        h1m1p = ps.tile([P, 1024], f32, tag="h1m1p")    # 2 banks
        h2m0 = ps.tile([P, 512], f32, tag="h2m0")
        h2m1 = ps.tile([P, 512], f32, tag="h2m1")
        # fmu2 [128, 1024]: rows 0:8 = f9 doses / piece vp, 64:66 = f9 base,
        # 32:41 = mu; cols half*512 per chunk-in-pair
        fmu2 = ps.tile([P, 1024], f32, tag="fmu2")

        # ======== small consts to SBUF ========
        icol = sb.tile([P, 1], f32)
        nc.sync.dma_start(out=icol[:], in_=icol_c[:])
        qcol = sb.tile([P, 1], f32)
        nc.sync.dma_start(out=qcol[:], in_=qcol_c[:])
        grp_rd = sb.tile([P, NG], bf16)
        nc.sync.dma_start(out=grp_rd[:], in_=grp_rd_c[:])
        ones100 = sb.tile([1, 100], bf16)
        nc.sync.dma_start(out=ones100[:], in_=ones100_c[:])
        L8 = sb.tile([NDOSES - 1, NDOSES], bf16)
        nc.sync.dma_start(out=L8[:], in_=L8_c[:])
        ones19 = sb.tile([1, NDOSES], bf16)
        nc.sync.dma_start(out=ones19[:], in_=ones19_c[:])
        ramp = sb.tile([NDOSES, 1], f32)
        nc.sync.dma_start(out=ramp[:], in_=ramp_c[:])
        ident = sb.tile([100, 100], f32)
        nc.sync.dma_start(out=ident[:], in_=ident_c[:])
        half_col = sb.tile([NDOSES - 1, 1], f32)
        nc.vector.memset(half_col[:], 0.5)


        # ======== index / table DMAs ========
        u_idx_sb = sb.tile([P, GS // 16], u16)
        nc.sync.dma_start(out=u_idx_sb[:], in_=u_idx[:])
        u_tidx_sb = sb.tile([P, GS // 16], u16)
        nc.sync.dma_start(out=u_tidx_sb[:], in_=u_tidx[:])

        cs_slab = sb.tile([P, SLAB], u8)
        dm_slab = sb.tile([P, SLAB], u8)
        for (slab, tab, eng) in ((cs_slab, cs8, nc.sync), (dm_slab, dm8, nc.gpsimd)):
            for h in range(2):  # split each table load across 2 issues
                eng.dma_start(
                    out=slab[h * 64:(h + 1) * 64, :],
                    in_=bass.AP(tensor=tab.ap().tensor, offset=0,
                                ap=[[0, 4], [SLAB, 16], [1, SLAB]]))

        # q values (idx >> 14) broadcast to each 16-partition group
        qbc_cs = sb.tile([P, GS], bf16)
        qbc_dm = sb.tile([P, GS], bf16)
        for (t_, row) in ((qbc_cs, qrow_cs), (qbc_dm, qrow_dm)):
            nc.scalar.dma_start(
                out=t_[:],
                in_=bass.AP(tensor=row.ap().tensor, offset=0,
                            ap=[[GS, NG], [0, 16], [1, GS]]))

        # ======== weights to SBUF ========
        # W1 (fp8, DoubleRow rhs layout): per kt a [128, 2, 1024] view
        w1t = []
        for kt in range(4):
            t_ = sb.tile([P, 2 * CEMB], fp8, tag=f"w1t_{kt}", name=f"w1t_{kt}")
            (nc.sync if kt % 2 == 0 else nc.scalar).dma_start(
                out=t_[:],
                in_=bass.AP(tensor=w18.ap().tensor, offset=kt * 256 * CEMB,
                            ap=[[CEMB, P], [P * CEMB, 2], [1, CEMB]]))
            w1t.append(t_)
        # cfT (fp8 DR lhsT): one [128, 4*2*100] tile, [p, (kt, t, c)]
        cft = sb.tile([P, 4 * 2 * 112], fp8)
        nc.gpsimd.dma_start(
            out=cft[:].rearrange("p (kt t c) -> p kt t c", kt=4, t=2)[:, :, :, 0:100],
            in_=bass.AP(tensor=cf8.ap().tensor, offset=0,
                        ap=[[100, P], [256 * 100, 4], [P * 100, 2], [1, 100]]))
        b1row = sb.tile([1, CEMB], bf16)
        nc.scalar.dma_start(out=b1row[:], in_=b1S[:].rearrange("(one n) -> one n", one=1))
        # Wf1c bf16: two [128, 4, 200] tiles (kt-major)
        wf1c_t = []
        for h in range(2):
            t_ = sb.tile([P, 4 * HID], bf16, tag=f"wf1c_{h}", name=f"wf1c_{h}")
            nc.scalar.dma_start(
                out=t_[:],
                in_=bass.AP(tensor=wf1c.ap().tensor, offset=h * 4 * P * HID,
                            ap=[[HID, P], [P * HID, 4], [1, HID]]))
            wf1c_t.append(t_)
        bf1b = sb.tile([P, HID], bf16)
        nc.scalar.dma_start(
            out=bf1b[:],
            in_=bass.AP(tensor=bf1S1.ap().tensor, offset=0, ap=[[0, P], [1, HID]]))
        me_sb = sb.tile([100, CEMB], f32)
        nc.scalar.dma_start(out=me_sb[:], in_=me_in[:])
        de2 = sb.tile([P, 2 * DEMB], bf16)  # [p, (mt, f)]
        nc.scalar.dma_start(
            out=de2[:],
            in_=bass.AP(tensor=de_bf.ap().tensor, offset=0,
                        ap=[[DEMB, P], [P * DEMB, 2], [1, DEMB]]))
        deT_sb = sb.tile([DEMB, NDRUG], bf16)
        nc.scalar.dma_start(out=deT_sb[:], in_=deT_bf[:])
        wf1d_sb = sb.tile([DEMB, HID], bf16)
        nc.scalar.dma_start(out=wf1d_sb[:], in_=wf1d[:])
        wf28_sb = sb.tile([P, 2 * 208], fp8)
        nc.sync.dma_start(out=wf28_sb[:], in_=wf28[:])
        wf38_sb = sb.tile([P, 2 * 16], fp8)
        nc.sync.dma_start(out=wf38_sb[:], in_=wf38[:])

        # ======== static chunk tiles + pad memsets ========
        # pair tiles: layout [p, (half, t, j)] = [128, 2048]
        h18 = [sb.tile([P, 2048], fp8, tag=f"h18_{i}", name=f"h18_{i}") for i in range(2)]
        h28 = [sb.tile([P, 2048], fp8, tag=f"h28_{i}", name=f"h28_{i}") for i in range(2)]
        for i in range(2):
            for hh in range(2):
                t1c = hh * 1024 + 512
                nc.vector.memset(h18[i][64:P, t1c:t1c + 512], 0.0)
                nc.vector.memset(h18[i][96:97, t1c:t1c + 512], S_H)
                nc.gpsimd.memset(h28[i][64:P, t1c:t1c + 512], 0.0)
                nc.gpsimd.memset(h28[i][96:97, t1c:t1c + 512], S_H2)
        a8 = sb.tile([P, 2 * 208], fp8)
        nc.vector.memset(a8[64:P, 208:416], 0.0)
        bd8 = sb.tile([P, 2 * 208], fp8)
        sc8 = [sb.tile([P, 2048], fp8, tag=f"sc8_{i}", name=f"sc8_{i}") for i in range(2)]
        sd8 = [sb.tile([P, 2048], fp8, tag=f"sd8_{i}", name=f"sd8_{i}") for i in range(2)]
        bc_t = [sb.tile([P, 1024], bf16, tag=f"bc_{i}", name=f"bc_{i}") for i in range(4)]
        bd_t = [sb.tile([P, 1024], bf16, tag=f"bd_{i}", name=f"bd_{i}") for i in range(4)]
        gb8 = [sb.tile([NDOSES - 1, 1024], bf16, tag=f"gb_{i}", name=f"gb_{i}") for i in range(2)]
        base_sc = [sb.tile([1, 1024], bf16, tag=f"base_{i}", name=f"base_{i}") for i in range(2)]
        spf = [sb.tile([NDOSES - 1, 1024], f32, tag=f"spf_{i}", name=f"spf_{i}") for i in range(2)]
        mu_sb = sb.tile([NDOSES, BS], f32)

        # piece tiles
        g_cs = sb.tile([P, GS], u8)
        g_dm = sb.tile([P, GS], u8)
        qm_t = [sb.tile([P, 512], bf16, tag=f"qm_{i}", name=f"qm_{i}") for i in range(2)]
        gtb_t = [sb.tile([P, 512], bf16, tag=f"gtb_{i}", name=f"gtb_{i}") for i in range(2)]
        v8_t = [sb.tile([NG, 512], bf16, tag=f"v8_{i}", name=f"v8_{i}") for i in range(2)]

        # ======== piece part 1: gathers + masks ========
        def emit_piece_gather(t):
            tsl = slice(t * 512, (t + 1) * 512)
            isl = slice(t * 32, (t + 1) * 32)
            for (k, gt, slab, ut, qbc) in (
                    (0, g_cs, cs_slab, u_idx_sb, qbc_cs),
                    (1, g_dm, dm_slab, u_tidx_sb, qbc_dm)):
                nc.gpsimd.indirect_copy(
                    out=gt[:, tsl].rearrange("p (n one) -> p n one", one=1),
                    data=slab[:], idxs=ut[:, isl],
                    i_know_ap_gather_is_preferred=True)
                nc.vector.tensor_scalar(
                    out=qm_t[k][:], in0=qbc[:, tsl], scalar1=qcol[:],
                    scalar2=None, op0=ALU.is_equal)
                nc.gpsimd.tensor_copy(out=gtb_t[k][:], in_=gt[:, tsl])
                nc.vector.tensor_tensor(
                    out=gtb_t[k][:], in0=gtb_t[k][:], in1=qm_t[k][:], op=ALU.mult)

        # ======== piece part 2: group-reduce + rowd store ========
        def emit_piece_reduce(t):
            for (k, rowd, bank) in ((0, cs_rowd, 0), (1, dm_rowd, 1)):
                vp = fmu2[0:NG, bank * 512:(bank + 1) * 512]
                nc.tensor.matmul(out=vp, lhsT=grp_rd[:], rhs=gtb_t[k][:],
                                 start=True, stop=True)
                nc.scalar.activation(out=v8_t[k][:], in_=vp, func=AF.Copy)
                nc.scalar.dma_start(
                    out=bass.AP(tensor=rowd.ap().tensor, offset=t * 512,
                                ap=[[GS, NG], [1, 512]]),
                    in_=v8_t[k][:])

        emit_piece_gather(0)

        # ======== P100 = relu(cf @ W1 + b1), scaled fp8 DR ========
        p_bf = sb.tile([100, CEMB], f32)
        for nh in range(2):
            pps = h1m0p[0:100, nh * 512:(nh + 1) * 512]
            for kt in range(4):
                nc.tensor.matmul(
                    out=pps,
                    lhsT=cft[:].rearrange("p (kt t c) -> p kt t c", kt=4, t=2)[:, kt, :, 0:100],
                    rhs=w1t[kt][:].rearrange("p (t n) -> p t n", t=2)[:, :, nh * 512:(nh + 1) * 512],
                    start=(kt == 0), stop=False, perf_mode=DR)
            nc.tensor.matmul(
                out=pps, lhsT=ones100[:], rhs=b1row[:, nh * 512:(nh + 1) * 512],
                start=False, stop=True)
        nc.scalar.activation(out=p_bf[:], in_=h1m0p[0:100, :],
                             func=AF.Relu, scale=1.0 / SP100)

        emit_piece_reduce(0)

        # ======== norms (squared-sum via activation accum) ========
        sq_scr = sb.tile([100, CEMB], bf16)  # discarded; only accum_out matters
        ssp = sb.tile([100, 1], f32)
        ssm = sb.tile([100, 1], f32)
        nc.scalar.activation(out=sq_scr[:], in_=p_bf[:], func=AF.Square,
                             accum_out=ssp[:])
        nc.scalar.activation(out=sq_scr[:], in_=me_sb[:], func=AF.Square,
                             accum_out=ssm[:])
        rd2 = sb.tile([P, 2], f32)
        sqd_scr = sb.tile([P, DEMB], bf16)
        for mt in range(2):
            nc.scalar.activation(out=sqd_scr[:], in_=de2[:, mt * DEMB:(mt + 1) * DEMB],
                                 func=AF.Square, accum_out=rd2[:, mt:mt + 1])
        for ss in (ssp, ssm):
            nc.scalar.activation(out=ss[:], in_=ss[:], func=AF.Sqrt)
            nc.vector.tensor_scalar_max(out=ss[:], in0=ss[:], scalar1=EPS)
            nc.vector.reciprocal(out=ss[:], in_=ss[:])
        nc.scalar.activation(out=rd2[:], in_=rd2[:], func=AF.Sqrt)
        nc.vector.tensor_scalar_max(out=rd2[:], in0=rd2[:], scalar1=EPS)
        nc.vector.reciprocal(out=rd2[:], in_=rd2[:])
        # rnS[mt] = S1 * rnorm for A m-tiles (states on partitions);
        # assembled with sbuf-to-sbuf DMAs (no partition-alignment limits)
        nc.vector.tensor_scalar_mul(out=ssp[:], in0=ssp[:], scalar1=S1)
        nc.vector.tensor_scalar_mul(out=ssm[:], in0=ssm[:], scalar1=S1)
        rn_m0 = sb.tile([P, 1], f32)
        rn_m1 = sb.tile([HID - P, 1], f32)
        nc.sync.dma_start(out=rn_m0[0:100, :], in_=ssp[:])
        nc.sync.dma_start(out=rn_m0[100:P, :], in_=ssm[0:28, :])
        nc.sync.dma_start(out=rn_m1[:], in_=ssm[28:100, :])

        # ======== cell table transpose: cnt_kt [128, 200] bf16 ========
        cnt_kt = []
        for kt in range(8):
            t_ = sb.tile([P, 2 * 100], bf16, tag=f"cnt_{kt}")
            for (ci, (src, co)) in enumerate(((p_bf, 0), (me_sb, 100))):
                tp = h1m1p[:, ((2 * kt + ci) % 2) * 512:((2 * kt + ci) % 2) * 512 + 100]
                nc.tensor.transpose(
                    out=tp, in_=src[:, kt * P:(kt + 1) * P], identity=ident[:])
                if (kt + ci) % 2 == 0:
                    nc.vector.tensor_copy(out=t_[:, co:co + 100], in_=tp)
                else:
                    nc.scalar.activation(out=t_[:, co:co + 100], in_=tp,
                                         func=AF.Copy)
            cnt_kt.append(t_)

        # ======== A (states x hid) -> a8 fp8 DR lhsT ========
        t1_scr = sb.tile([P, HID], bf16)
        for (mt, msl, mm, rn) in ((0, slice(0, P), P, rn_m0),
                                  (1, slice(P, HID), HID - P, rn_m1)):
            aps = (h2m0 if mt == 0 else h2m1)[0:mm, 0:HID]
            for kt in range(8):
                nc.tensor.matmul(
                    out=aps,
                    lhsT=cnt_kt[kt][:, msl],
                    rhs=wf1c_t[kt // 4][:].rearrange("p (k m) -> p k m", k=4)[:, kt % 4],
                    start=(kt == 0), stop=(kt == 7))
            nc.vector.tensor_scalar_mul(out=t1_scr[0:mm, :], in0=aps, scalar1=rn[:])
            nc.vector.tensor_tensor(
                out=a8[0:mm, mt * 208:mt * 208 + HID], in0=t1_scr[0:mm, :],
                in1=bf1b[0:mm, :], op=ALU.add)

        # ======== Bd (drugs x hid) -> bd8 fp8 DR lhsT ========
        rdS = sb.tile([P, 2], f32)
        nc.vector.tensor_scalar_mul(out=rdS[:], in0=rd2[:], scalar1=S1)
        for mt in range(2):
            bps = (h2m0 if mt == 0 else h2m1)[:, 0:HID]
            nc.tensor.matmul(out=bps, lhsT=deT_sb[:, mt * P:(mt + 1) * P],
                             rhs=wf1d_sb[:], start=True, stop=True)
            nc.scalar.activation(out=bd8[:, mt * 208:mt * 208 + HID], in_=bps,
                                 func=AF.Copy, scale=rdS[:, mt:mt + 1])

        # ======== chunk pipeline (pair-fused: 2 chunks per emit) ========
        def emit_dma_pair(c0, bb):
            # issue the cs/dm broadcast loads for pair (c0, c0+2) into buffer bb
            n0 = c0 * 512
            for (dst, rowd, eng) in ((bc_t[bb], cs_rowd, nc.sync),
                                     (bd_t[bb], dm_rowd, nc.gpsimd)):
                for hh in range(2):
                    eng.dma_start(
                        out=dst[:, hh * 512:(hh + 1) * 512],
                        in_=bass.AP(tensor=rowd.ap().tensor,
                                    offset=n0 + hh * 1024,
                                    ap=[[0, P], [1, 512]]))

        def emit_pair(c0, pp, bb):
            # chunks c0 and c0+2 (same piece parity); halves 0/1 of pair tiles
            n0 = c0 * 512
            # one-hot layout [p, (t, half, j)]: plane t built with one
            # [128, 1024] tensor-scalar over both halves
            nc.vector.tensor_scalar(out=sc8[pp][:, 0:1024], in0=bc_t[bb][:],
                                    scalar1=icol[:], scalar2=None,
                                    op0=ALU.is_equal)
            nc.vector.tensor_scalar(out=sc8[pp][:, 1024:2048], in0=bc_t[bb][:],
                                    scalar1=128.0, scalar2=icol[:],
                                    op0=ALU.subtract, op1=ALU.is_equal)
            nc.vector.tensor_scalar(out=sd8[pp][:, 0:1024], in0=bd_t[bb][:],
                                    scalar1=icol[:], scalar2=None,
                                    op0=ALU.is_equal)
            nc.vector.tensor_scalar(out=sd8[pp][:, 1024:2048], in0=bd_t[bb][:],
                                    scalar1=128.0, scalar2=icol[:],
                                    op0=ALU.subtract, op1=ALU.is_equal)
            a8_v = a8[:].rearrange("p (t m) -> p t m", t=2)
            bd8_v = bd8[:].rearrange("p (t m) -> p t m", t=2)
            # h1 matmuls per half into the 2-bank pair psum
            for hh in range(2):
                sc_v = sc8[pp][:].rearrange("p (t h n) -> p t h n", t=2, h=2)[:, :, hh, :]
                sd_v = sd8[pp][:].rearrange("p (t h n) -> p t h n", t=2, h=2)[:, :, hh, :]
                hsl = slice(hh * 512, (hh + 1) * 512)
                nc.tensor.matmul(out=h1m0p[:, hsl], lhsT=a8_v[:, :, 0:P],
                                 rhs=sc_v, start=True, stop=False, perf_mode=DR)
                nc.tensor.matmul(out=h1m0p[:, hsl], lhsT=bd8_v[:, :, 0:P],
                                 rhs=sd_v, start=False, stop=True, perf_mode=DR)
                nc.tensor.matmul(out=h1m1p[0:HID - P, hsl],
                                 lhsT=a8_v[:, :, P:HID],
                                 rhs=sc_v, start=True, stop=False, perf_mode=DR)
                nc.tensor.matmul(out=h1m1p[0:HID - P, hsl],
                                 lhsT=bd8_v[:, :, P:HID],
                                 rhs=sd_v, start=False, stop=True, perf_mode=DR)
            # pair-fused relus: psum [*, 1024] -> strided fp8 [p, (h, t0/t1, j)]
            h18_v = h18[pp][:].rearrange("p (h t n) -> p h t n", h=2, t=2)
            nc.scalar.activation(out=h18_v[:, :, 0, :], in_=h1m0p[:],
                                 func=AF.Relu, scale=S_H / S1)
            nc.vector.tensor_scalar(out=h18_v[0:HID - P, :, 1, :],
                                    in0=h1m1p[0:HID - P, :],
                                    scalar1=S_H / S1, scalar2=0.0,
                                    op0=ALU.mult, op1=ALU.max)
            # h2 per half (single-bank psums), relus write (h, t) lanes
            w2_v = wf28_sb[:].rearrange("p (t m) -> p t m", t=2)
            h28_v = h28[pp][:].rearrange("p (h t n) -> p h t n", h=2, t=2)
            for hh in range(2):
                h1_v = h18[pp][:].rearrange("p (h t n) -> p h t n", h=2, t=2)[:, hh]
                nc.tensor.matmul(out=h2m0[:], lhsT=w2_v[:, :, 0:P], rhs=h1_v,
                                 start=True, stop=True, perf_mode=DR)
                nc.tensor.matmul(out=h2m1[0:HID - P, :], lhsT=w2_v[:, :, P:HID],
                                 rhs=h1_v, start=True, stop=True, perf_mode=DR)
                nc.scalar.activation(out=h28_v[:, hh, 0, :], in_=h2m0[:],
                                     func=AF.Relu, scale=S_H2 / (S_H * S_W2))
                nc.scalar.activation(out=h28_v[0:HID - P, hh, 1, :],
                                     in_=h2m1[0:HID - P, :],
                                     func=AF.Relu, scale=S_H2 / (S_H * S_W2))
            # fwd per half: doses to fmu2[0:8, half], base to fmu2[64:66, half]
            w3_v = wf38_sb[:].rearrange("p (t m) -> p t m", t=2)
            for hh in range(2):
                h2_v = h28[pp][:].rearrange("p (h t n) -> p h t n", h=2, t=2)[:, hh]
                hsl = slice(hh * 512, (hh + 1) * 512)
                nc.tensor.matmul(out=fmu2[0:NDOSES - 1, hsl],
                                 lhsT=w3_v[:, :, 0:NDOSES - 1], rhs=h2_v,
                                 start=True, stop=True, perf_mode=DR)
                nc.tensor.matmul(out=fmu2[64:66, hsl],
                                 lhsT=w3_v[:, 0, NDOSES - 1:NDOSES + 1],
                                 rhs=h2_v[:, 0, :], start=True, stop=False)
                nc.tensor.matmul(out=fmu2[64:66, hsl],
                                 lhsT=w3_v[:, 1, NDOSES - 1:NDOSES + 1],
                                 rhs=h2_v[:, 1, :], start=False, stop=True)
            # pair-fused tail: softplus-ln2 = ln(0.5 exp + 0.5); base scale
            nc.scalar.activation(out=spf[pp][:], in_=fmu2[0:NDOSES - 1, :],
                                 func=AF.Exp, scale=1.0 / (S_H2 * S_W3))
            nc.scalar.activation(out=gb8[pp][:], in_=spf[pp][:],
                                 func=AF.Ln, scale=0.5, bias=half_col[:])
            nc.vector.tensor_scalar_mul(out=base_sc[pp][:], in0=fmu2[64:65, :],
                                        scalar1=1.0 / (S_H2 * S_W3))
            # mu per half: base + cumsum via two matmuls into fmu2[32:41]
            for hh in range(2):
                hsl = slice(hh * 512, (hh + 1) * 512)
                nc.tensor.matmul(out=fmu2[32:32 + NDOSES, hsl], lhsT=L8[:],
                                 rhs=gb8[pp][:, hsl], start=True, stop=False)
                nc.tensor.matmul(out=fmu2[32:32 + NDOSES, hsl], lhsT=ones19[:],
                                 rhs=base_sc[pp][:, hsl], start=False, stop=True)
            # pair-fused ramp add; out columns of chunks c0 and c0+2
            nc.vector.tensor_scalar(
                out=bass.AP(tensor=mu_sb[:].tensor, offset=n0,
                            ap=[[BS, NDOSES], [1024, 2], [1, 512]]),
                in0=fmu2[32:32 + NDOSES, :],
                scalar1=ramp[:], scalar2=None, op0=ALU.add)

        evens = (0, 4, 8, 12)
        odds = (1, 5, 9, 13)
        for i, c0 in enumerate(evens):
            emit_dma_pair(c0, i % 4)
        for i, c0 in enumerate(evens):
            emit_pair(c0, i % 2, i % 4)
        emit_piece_gather(1)
        emit_piece_reduce(1)
        for i, c0 in enumerate(odds):
            emit_dma_pair(c0, i % 4)
        for i, c0 in enumerate(odds):
            emit_pair(c0, i % 2, i % 4)

        nc.sync.dma_start(out=mu_s[:], in_=mu_sb[:])

    return _split_sync_waits(nc) if split_waits else nc


def _get_nc():
    if "nc" not in _NC_CACHE:
        _NC_CACHE["nc"] = build_nc()
    return _NC_CACHE["nc"]


def _wrap16(vals):
    # vals [8192] in sample order k (g = k>>10, j = k&1023)
    # -> [128, 64] at [16g + (j & 15), j >> 4]
    v = vals.reshape(NG, GS // 16, 16)        # [g, j_hi, j_lo]
    v = np.transpose(v, (0, 2, 1))            # [g, j_lo, j_hi]
    return np.ascontiguousarray(v.reshape(P, GS // 16))


def make_in_maps(inputs):
    idx = np.asarray(inputs["idx"], np.int64)
    tidx = np.asarray(inputs["tidx"], np.int64)
    cf = np.asarray(inputs["cell_features"], np.float32)
    me = np.asarray(inputs["missing_emb"], np.float32)
    de = np.asarray(inputs["drug_emb"], np.float32)
    W1 = np.asarray(inputs["W1"], np.float32)
    b1 = np.asarray(inputs["b1"], np.float32)
    Wf1 = np.asarray(inputs["Wf1"], np.float32)
    bf1 = np.asarray(inputs["bf1"], np.float32)
    Wf2 = np.asarray(inputs["Wf2"], np.float32)
    bf2 = np.asarray(inputs["bf2"], np.float32)
    Wf3 = np.asarray(inputs["Wf3"], np.float32)
    bf3 = np.asarray(inputs["bf3"], np.float32)

    cs_full = (np.asarray(inputs["cell_map"], np.int64)
               + 100 * np.asarray(inputs["is_missing"], np.int64))

    # Wf2/Wf3 fp8 DR lhsT with the bias folded into pad row (t=1, p=96);
    # Wf3 columns permuted to [dose1..dose8, dose0(base)]
    perm = [1, 2, 3, 4, 5, 6, 7, 8, 0]
    Wf3p = Wf3[:, perm]
    bf3p = bf3[perm]
    wf28 = np.zeros((P, 2, 208), NP_FP8)
    wf28[:, 0, :HID] = (Wf2[0:P, :] * S_W2).astype(NP_FP8)
    wf28[:HID - P, 1, :HID] = (Wf2[P:HID, :] * S_W2).astype(NP_FP8)
    wf28[96, 1, :HID] = (bf2 * S_W2).astype(NP_FP8)
    Wf3d = np.concatenate([Wf3p, Wf3p[:, -1:]], axis=1)   # duplicate base col
    bf3d = np.concatenate([bf3p, bf3p[-1:]])
    wf38 = np.zeros((P, 2, 16), NP_FP8)
    wf38[:, 0, :NDOSES + 1] = (Wf3d[0:P, :] * S_W3).astype(NP_FP8)
    wf38[:HID - P, 1, :NDOSES + 1] = (Wf3d[P:HID, :] * S_W3).astype(NP_FP8)
    wf38[96, 1, :NDOSES + 1] = (bf3d * S_W3).astype(NP_FP8)

    shared = dict(
        cs8=np.ascontiguousarray(cs_full.astype(np.uint8)),
        dm8=np.ascontiguousarray(np.asarray(inputs["drug_map"]).astype(np.uint8)),
        w18=np.ascontiguousarray((W1 * S_W1).astype(NP_FP8)),
        cf8=np.ascontiguousarray((cf[:100, :].T * S_CF).astype(NP_FP8)),
        b1S=np.ascontiguousarray((b1 * SP100).astype(NP_BF16)),
        wf1c=np.ascontiguousarray(Wf1[:CEMB, :].astype(NP_BF16)),
        bf1S1=np.ascontiguousarray((bf1 * S1).astype(NP_BF16)),
        me_in=np.ascontiguousarray(me),
        de_bf=np.ascontiguousarray(de.astype(NP_BF16)),
        deT_bf=np.ascontiguousarray(de.T.astype(NP_BF16)),
        wf1d=np.ascontiguousarray(Wf1[CEMB:, :].astype(NP_BF16)),
        wf28=np.ascontiguousarray(wf28.reshape(P, 2 * 208)),
        wf38=np.ascontiguousarray(wf38.reshape(P, 2 * 16)),
    )

    in_maps = []
    for c in range(NCORES):
        ic = idx[c * BS:(c + 1) * BS]
        tc_ = tidx[c * BS:(c + 1) * BS]
        m = dict(shared)
        m["u_idx"] = _wrap16((ic & (SLAB - 1)).astype(np.uint16))
        m["u_tidx"] = _wrap16((tc_ & (SLAB - 1)).astype(np.uint16))
        m["qrow_cs"] = np.ascontiguousarray((ic >> 14).astype(NP_BF16))
        m["qrow_dm"] = np.ascontiguousarray((tc_ >> 14).astype(NP_BF16))
        in_maps.append(m)
    return in_maps


def kernel(**inputs):
    nc = _get_nc()
    in_maps = make_in_maps(inputs)
    last_err = None
    for _attempt in range(3):
        try:
            res = run_bass_kernel_spmd(nc, in_maps, core_ids=list(range(NCORES)))
            return np.ascontiguousarray(np.concatenate(
                [res.results[c]["mu_s"].T for c in range(NCORES)], axis=0))
        except Exception as e:  # wedged device sometimes recovers on retry
            last_err = e
    raise last_err


# revision 31
# speedup vs baseline: 1.2327x; 1.2327x over previous
"""Trainium2 Bass kernel for nn_DrugResponsePrior (embedding_lookup).

Spec guarantees: cell_map < 100, is_missing in {0,1}, drug_map < 256.  So each
row depends only on cs = cell_map[idx]+100*is_missing[idx] (200 states) and
dm = drug_map[tidx] (256 drugs).

Fully data-parallel (8 cores x 8192 samples, no collectives), fp8 pipeline:
  1. cs/dm u8 tables in a 16-slab SBUF layout; per-sample lookup with gpsimd
     indirect_copy + one-hot slab mask + group-reduce matmul (bf16).
  2. Tables A = l2n(cell table) @ Wf1c + bf1 (200x200) and Bd = l2n(drug_emb)
     @ Wf1d (256x200) built on device, stored as scaled fp8 DoubleRow lhsT.
  3. Per 512-sample chunk: one-hot matrices (fp8) over cs/dm, h1/h2/fwd as
     fp8 DoubleRow matmuls (0.5 cyc/row), biases folded into pad rows or
     activation bias, softplus residual + cumsum via one bf16 9x9 matmul plus
     an exact f32 ln2-ramp add.
Output mu is written [9, BS] per core; host transposes/concats.
Numerics validated on host: rel_fro ~ 5e-4 (gate 2e-2) with these scales.
"""
import sys

if "/opt/trn_rl_repo" not in sys.path:
    sys.path.insert(0, "/opt/trn_rl_repo")

import numpy as np
import ml_dtypes

import concourse.bass as bass
import concourse.mybir as mybir
import concourse.tile as tile
from concourse.bass_utils import run_bass_kernel_spmd

f32 = mybir.dt.float32
bf16 = mybir.dt.bfloat16
fp8 = mybir.dt.float8e4
u16 = mybir.dt.uint16
u8 = mybir.dt.uint8

NP_BF16 = ml_dtypes.bfloat16
NP_FP8 = ml_dtypes.float8_e4m3

B = 65536
R = 262144
NDRUG = 256
NFEAT = 1024
CEMB = 1024
DEMB = 128
HID = 200
NDOSES = 9
NCORES = 8

BS = B // NCORES            # 8192 samples per core
P = 128
NG = 8                      # index groups (16 partitions each)
GS = BS // NG               # 1024 samples per group
SLAB = R // 16              # 16384 entries per slab partition
NCHUNK = BS // 512          # 16 chunks of 512 samples
EPS = 1e-12
LN2 = float(np.log(2.0))

# power-of-two scales for fp8 operands (validated on host: rel_fro ~ 5e-4)
S_CF = 8.0                  # cell_features
S_W1 = 64.0                 # W1
SP100 = S_CF * S_W1         # psum scale of cf @ W1
S1 = 256.0                  # A / Bd tables (h1 psum scale)
S_H = 64.0                  # h1 activations
S_W2 = 256.0                # Wf2
S_H2 = 64.0                 # h2 activations
S_W3 = 1024.0               # Wf3

_NC_CACHE = {}


def _split_sync_waits(nc, limit=1):
    """walrus accepts at most one sync-wait per instruction; hoist excess
    waits onto same-engine NoOps inserted just before."""
    ctr = 0
    for bb in nc.main_func.blocks:
        new_list = []
        for inst in bb.instructions:
            si = inst.sync_info
            if si is not None and si.on_wait and len(si.on_wait) > limit:
                waits = list(si.on_wait)
                head, tail = waits[:-limit], waits[-limit:]
                for j in range(0, len(head), limit):
                    nop = mybir.InstNoOp(name=f"waitnop-{ctr}", engine=inst.engine)
                    ctr += 1
                    nop.sync_info = mybir.SyncInfo(
                        on_wait=list(head[j : j + limit]), on_update=[]
                    )
                    new_list.append(nop)
                inst.sync_info = mybir.SyncInfo(
                    on_wait=list(tail),
                    on_update=list(si.on_update) if si.on_update else [],
                )
            new_list.append(inst)
        bb.instructions[:] = new_list
    return nc


def build_nc(split_waits=True):
    nc = bass.Bass(num_devices=NCORES)
    AF = mybir.ActivationFunctionType
    ALU = mybir.AluOpType
    DR = mybir.MatmulPerfMode.DoubleRow

    # ---------------- kernel I/O ----------------
    cs8 = nc.dram_tensor("cs8", [R], u8, kind="ExternalInput")
    dm8 = nc.dram_tensor("dm8", [R], u8, kind="ExternalInput")
    u_idx = nc.dram_tensor("u_idx", [P, GS // 16], u16, kind="ExternalInput")
    u_tidx = nc.dram_tensor("u_tidx", [P, GS // 16], u16, kind="ExternalInput")
    qrow_cs = nc.dram_tensor("qrow_cs", [NG * GS], bf16, kind="ExternalInput")
    qrow_dm = nc.dram_tensor("qrow_dm", [NG * GS], bf16, kind="ExternalInput")
    w18 = nc.dram_tensor("w18", [NFEAT, CEMB], fp8, kind="ExternalInput")
    cf8 = nc.dram_tensor("cf8", [NFEAT, 100], fp8, kind="ExternalInput")
    b1S = nc.dram_tensor("b1S", [CEMB], bf16, kind="ExternalInput")
    wf1c = nc.dram_tensor("wf1c", [CEMB, HID], bf16, kind="ExternalInput")
    bf1S1 = nc.dram_tensor("bf1S1", [HID], bf16, kind="ExternalInput")
    me_in = nc.dram_tensor("me_in", [100, CEMB], f32, kind="ExternalInput")
    de_bf = nc.dram_tensor("de_bf", [NDRUG, DEMB], bf16, kind="ExternalInput")
    deT_bf = nc.dram_tensor("deT_bf", [DEMB, NDRUG], bf16, kind="ExternalInput")
    wf1d = nc.dram_tensor("wf1d", [DEMB, HID], bf16, kind="ExternalInput")
    wf28 = nc.dram_tensor("wf28", [P, 2 * 208], fp8, kind="ExternalInput")
    wf38 = nc.dram_tensor("wf38", [P, 2 * 32], fp8, kind="ExternalInput")
    mu_s = nc.dram_tensor("mu_s", [NDOSES, BS], f32, kind="ExternalOutput")

    # internal DRAM rows for the per-sample cs/dm values (broadcast reload)
    cs_rowd = nc.dram_tensor("cs_rowd", [BS], bf16)
    dm_rowd = nc.dram_tensor("dm_rowd", [BS], bf16)

    # inline constants
    icol_c = nc.inline_tensor(
        np.arange(P, dtype=np.float32).reshape(P, 1), name="icol_c")
    qcol_c = nc.inline_tensor(
        (np.arange(P, dtype=np.float32) % 16).reshape(P, 1), name="qcol_c")
    grp_rd_c = nc.inline_tensor(  # [128, 8] group-reduce lhsT (bf16)
        np.array([[1.0 if (k // 16) == g else 0.0 for g in range(NG)]
                  for k in range(P)], NP_BF16), name="grp_rd_c")
    ones100_c = nc.inline_tensor(np.ones((1, 100), NP_BF16), name="ones100_c")
    # L8S[k, o] = S_H2*S_W3 iff dose k+1 contributes to output o (k+1 <= o);
    # pre-scaled so it accumulates onto the (scaled) base-broadcast psum
    L8np = np.zeros((NDOSES - 1, NDOSES), NP_BF16)
    for k in range(NDOSES - 1):
        L8np[k, k + 1:] = S_H2 * S_W3
    L8S_c = nc.inline_tensor(L8np, name="L8S_c")
    ramp_c = nc.inline_tensor(
        (LN2 * np.arange(NDOSES, dtype=np.float32)).reshape(NDOSES, 1),
        name="ramp_c")
    ident_c = nc.inline_tensor(np.eye(100, dtype=np.float32), name="ident_c")


    with tile.TileContext(nc) as tc, \
            tc.tile_pool(name="sb", bufs=1) as sb, \
            tc.tile_pool(name="ps", bufs=1, space="PSUM") as ps:

        # ======== psum banks: pair-fused 2-bank tiles (8 banks total) ========
        h1m0p = ps.tile([P, 1024], f32, tag="h1m0p")    # 2 banks
        h1m1p = ps.tile([P, 1024], f32, tag="h1m1p")    # 2 banks
        h2m0 = ps.tile([P, 512], f32, tag="h2m0")
        h2m1 = ps.tile([P, 512], f32, tag="h2m1")
        # fmu2 [128, 1024]: rows 0:8 = f9 doses / piece vp, 64:66 = f9 base,
        # 32:41 = mu; cols half*512 per chunk-in-pair
        fmu2 = ps.tile([P, 1024], f32, tag="fmu2")

        # ======== small consts to SBUF ========
        icol = sb.tile([P, 1], f32)
        nc.sync.dma_start(out=icol[:], in_=icol_c[:])
        qcol = sb.tile([P, 1], f32)
        nc.sync.dma_start(out=qcol[:], in_=qcol_c[:])
        grp_rd = sb.tile([P, NG], bf16)
        nc.sync.dma_start(out=grp_rd[:], in_=grp_rd_c[:])
        ones100 = sb.tile([1, 100], bf16)
        nc.sync.dma_start(out=ones100[:], in_=ones100_c[:])
        L8S = sb.tile([NDOSES - 1, NDOSES], bf16)
        nc.sync.dma_start(out=L8S[:], in_=L8S_c[:])
        rs_col = sb.tile([NDOSES, 1], f32)
        nc.vector.memset(rs_col[:], 1.0 / (S_H2 * S_W3))
        ramp = sb.tile([NDOSES, 1], f32)
        nc.sync.dma_start(out=ramp[:], in_=ramp_c[:])
        ident = sb.tile([100, 100], f32)
        nc.sync.dma_start(out=ident[:], in_=ident_c[:])
        half_col = sb.tile([NDOSES - 1, 1], f32)
        nc.vector.memset(half_col[:], 0.5)


        # ======== index / table DMAs ========
        u_idx_sb = sb.tile([P, GS // 16], u16)
        nc.sync.dma_start(out=u_idx_sb[:], in_=u_idx[:])
        u_tidx_sb = sb.tile([P, GS // 16], u16)
        nc.sync.dma_start(out=u_tidx_sb[:], in_=u_tidx[:])

        cs_slab = sb.tile([P, SLAB], u8)
        dm_slab = sb.tile([P, SLAB], u8)
        for (slab, tab, eng) in ((cs_slab, cs8, nc.sync), (dm_slab, dm8, nc.gpsimd)):
            for h in range(2):  # split each table load across 2 issues
                eng.dma_start(
                    out=slab[h * 64:(h + 1) * 64, :],
                    in_=bass.AP(tensor=tab.ap().tensor, offset=0,
                                ap=[[0, 4], [SLAB, 16], [1, SLAB]]))

        # q values (idx >> 14) broadcast to each 16-partition group
        qbc_cs = sb.tile([P, GS], bf16)
        qbc_dm = sb.tile([P, GS], bf16)
        for (t_, row) in ((qbc_cs, qrow_cs), (qbc_dm, qrow_dm)):
            nc.scalar.dma_start(
                out=t_[:],
                in_=bass.AP(tensor=row.ap().tensor, offset=0,
                            ap=[[GS, NG], [0, 16], [1, GS]]))

        # ======== weights to SBUF ========
        # W1 (fp8, DoubleRow rhs layout): per kt a [128, 2, 1024] view
        w1t = []
        for kt in range(4):
            t_ = sb.tile([P, 2 * CEMB], fp8, tag=f"w1t_{kt}", name=f"w1t_{kt}")
            (nc.sync if kt % 2 == 0 else nc.scalar).dma_start(
                out=t_[:],
                in_=bass.AP(tensor=w18.ap().tensor, offset=kt * 256 * CEMB,
                            ap=[[CEMB, P], [P * CEMB, 2], [1, CEMB]]))
            w1t.append(t_)
        # cfT (fp8 DR lhsT): one [128, 4*2*100] tile, [p, (kt, t, c)]
        cft = sb.tile([P, 4 * 2 * 112], fp8)
        nc.gpsimd.dma_start(
            out=cft[:].rearrange("p (kt t c) -> p kt t c", kt=4, t=2)[:, :, :, 0:100],
            in_=bass.AP(tensor=cf8.ap().tensor, offset=0,
                        ap=[[100, P], [256 * 100, 4], [P * 100, 2], [1, 100]]))
        b1row = sb.tile([1, CEMB], bf16)
        nc.scalar.dma_start(out=b1row[:], in_=b1S[:].rearrange("(one n) -> one n", one=1))
        # Wf1c bf16: two [128, 4, 200] tiles (kt-major)
        wf1c_t = []
        for h in range(2):
            t_ = sb.tile([P, 4 * HID], bf16, tag=f"wf1c_{h}", name=f"wf1c_{h}")
            nc.scalar.dma_start(
                out=t_[:],
                in_=bass.AP(tensor=wf1c.ap().tensor, offset=h * 4 * P * HID,
                            ap=[[HID, P], [P * HID, 4], [1, HID]]))
            wf1c_t.append(t_)
        bf1b = sb.tile([P, HID], bf16)
        nc.scalar.dma_start(
            out=bf1b[:],
            in_=bass.AP(tensor=bf1S1.ap().tensor, offset=0, ap=[[0, P], [1, HID]]))
        me_sb = sb.tile([100, CEMB], f32)
        nc.scalar.dma_start(out=me_sb[:], in_=me_in[:])
        de2 = sb.tile([P, 2 * DEMB], bf16)  # [p, (mt, f)]
        nc.scalar.dma_start(
            out=de2[:],
            in_=bass.AP(tensor=de_bf.ap().tensor, offset=0,
                        ap=[[DEMB, P], [P * DEMB, 2], [1, DEMB]]))
        deT_sb = sb.tile([DEMB, NDRUG], bf16)
        nc.scalar.dma_start(out=deT_sb[:], in_=deT_bf[:])
        wf1d_sb = sb.tile([DEMB, HID], bf16)
        nc.scalar.dma_start(out=wf1d_sb[:], in_=wf1d[:])
        wf28_sb = sb.tile([P, 2 * 208], fp8)
        nc.sync.dma_start(out=wf28_sb[:], in_=wf28[:])
        wf38_sb = sb.tile([P, 2 * 32], fp8)
        nc.sync.dma_start(out=wf38_sb[:], in_=wf38[:])

        # ======== static chunk tiles + pad memsets ========
        # pair tiles: layout [p, (half, t, j)] = [128, 2048]
        h18 = [sb.tile([P, 2048], fp8, tag=f"h18_{i}", name=f"h18_{i}") for i in range(2)]
        h28 = [sb.tile([P, 2048], fp8, tag=f"h28_{i}", name=f"h28_{i}") for i in range(2)]
        for i in range(2):
            for hh in range(2):
                t1c = hh * 1024 + 512
                nc.vector.memset(h18[i][64:P, t1c:t1c + 512], 0.0)
                nc.vector.memset(h18[i][96:97, t1c:t1c + 512], S_H)
                nc.gpsimd.memset(h28[i][64:P, t1c:t1c + 512], 0.0)
                nc.gpsimd.memset(h28[i][96:97, t1c:t1c + 512], S_H2)
        a8 = sb.tile([P, 2 * 208], fp8)
        nc.vector.memset(a8[64:P, 208:416], 0.0)
        bd8 = sb.tile([P, 2 * 208], fp8)
        sc8 = [sb.tile([P, 2048], fp8, tag=f"sc8_{i}", name=f"sc8_{i}") for i in range(2)]
        sd8 = [sb.tile([P, 2048], fp8, tag=f"sd8_{i}", name=f"sd8_{i}") for i in range(2)]
        bc_t = [sb.tile([P, 1024], bf16, tag=f"bc_{i}", name=f"bc_{i}") for i in range(4)]
        bd_t = [sb.tile([P, 1024], bf16, tag=f"bd_{i}", name=f"bd_{i}") for i in range(4)]
        gb8 = [sb.tile([NDOSES - 1, 1024], bf16, tag=f"gb_{i}", name=f"gb_{i}") for i in range(2)]
        spf = [sb.tile([NDOSES - 1, 1024], f32, tag=f"spf_{i}", name=f"spf_{i}") for i in range(2)]
        mu_sb = sb.tile([NDOSES, BS], f32)

        # piece tiles
        g_cs = sb.tile([P, GS], u8)
        g_dm = sb.tile([P, GS], u8)
        qm_t = [sb.tile([P, 512], bf16, tag=f"qm_{i}", name=f"qm_{i}") for i in range(2)]
        gtb_t = [sb.tile([P, 512], bf16, tag=f"gtb_{i}", name=f"gtb_{i}") for i in range(2)]
        v8_t = [sb.tile([NG, 512], bf16, tag=f"v8_{i}", name=f"v8_{i}") for i in range(2)]

        # ======== piece part 1: gathers + masks ========
        def emit_piece_gather(t):
            tsl = slice(t * 512, (t + 1) * 512)
            isl = slice(t * 32, (t + 1) * 32)
            for (k, gt, slab, ut, qbc) in (
                    (0, g_cs, cs_slab, u_idx_sb, qbc_cs),
                    (1, g_dm, dm_slab, u_tidx_sb, qbc_dm)):
                nc.gpsimd.indirect_copy(
                    out=gt[:, tsl].rearrange("p (n one) -> p n one", one=1),
                    data=slab[:], idxs=ut[:, isl],
                    i_know_ap_gather_is_preferred=True)
                nc.vector.tensor_scalar(
                    out=qm_t[k][:], in0=qbc[:, tsl], scalar1=qcol[:],
                    scalar2=None, op0=ALU.is_equal)
                nc.scalar.activation(out=gtb_t[k][:], in_=gt[:, tsl], func=AF.Copy)
                nc.vector.tensor_tensor(
                    out=gtb_t[k][:], in0=gtb_t[k][:], in1=qm_t[k][:], op=ALU.mult)

        # ======== piece part 2: group-reduce + rowd store ========
        def emit_piece_reduce(t):
            for (k, rowd, bank) in ((0, cs_rowd, 0), (1, dm_rowd, 1)):
                vp = fmu2[0:NG, bank * 512:(bank + 1) * 512]
                nc.tensor.matmul(out=vp, lhsT=grp_rd[:], rhs=gtb_t[k][:],
                                 start=True, stop=True)
                nc.scalar.activation(out=v8_t[k][:], in_=vp, func=AF.Copy)
                nc.scalar.dma_start(
                    out=bass.AP(tensor=rowd.ap().tensor, offset=t * 512,
                                ap=[[GS, NG], [1, 512]]),
                    in_=v8_t[k][:])

        emit_piece_gather(0)

        # ======== P100 = relu(cf @ W1 + b1), scaled fp8 DR ========
        p_bf = sb.tile([100, CEMB], f32)
        for nh in range(2):
            pps = h1m0p[0:100, nh * 512:(nh + 1) * 512]
            for kt in range(4):
                nc.tensor.matmul(
                    out=pps,
                    lhsT=cft[:].rearrange("p (kt t c) -> p kt t c", kt=4, t=2)[:, kt, :, 0:100],
                    rhs=w1t[kt][:].rearrange("p (t n) -> p t n", t=2)[:, :, nh * 512:(nh + 1) * 512],
                    start=(kt == 0), stop=False, perf_mode=DR)
            nc.tensor.matmul(
                out=pps, lhsT=ones100[:], rhs=b1row[:, nh * 512:(nh + 1) * 512],
                start=False, stop=True)
        nc.scalar.activation(out=p_bf[:], in_=h1m0p[0:100, :],
                             func=AF.Relu, scale=1.0 / SP100)

        emit_piece_reduce(0)

        # ======== norms (squared-sum via activation accum) ========
        sq_scr = sb.tile([100, CEMB], bf16)  # discarded; only accum_out matters
        ssp = sb.tile([100, 1], f32)
        ssm = sb.tile([100, 1], f32)
        nc.scalar.activation(out=sq_scr[:], in_=p_bf[:], func=AF.Square,
                             accum_out=ssp[:])
        nc.scalar.activation(out=sq_scr[:], in_=me_sb[:], func=AF.Square,
                             accum_out=ssm[:])
        rd2 = sb.tile([P, 2], f32)
        sqd_scr = sb.tile([P, DEMB], bf16)
        for mt in range(2):
            nc.scalar.activation(out=sqd_scr[:], in_=de2[:, mt * DEMB:(mt + 1) * DEMB],
                                 func=AF.Square, accum_out=rd2[:, mt:mt + 1])
        for ss in (ssp, ssm):
            nc.scalar.activation(out=ss[:], in_=ss[:], func=AF.Sqrt)
            nc.vector.tensor_scalar_max(out=ss[:], in0=ss[:], scalar1=EPS)
            nc.vector.reciprocal(out=ss[:], in_=ss[:])
        nc.scalar.activation(out=rd2[:], in_=rd2[:], func=AF.Sqrt)
        nc.vector.tensor_scalar_max(out=rd2[:], in0=rd2[:], scalar1=EPS)
        nc.vector.reciprocal(out=rd2[:], in_=rd2[:])
        # rnS[mt] = S1 * rnorm for A m-tiles (states on partitions);
        # assembled with sbuf-to-sbuf DMAs (no partition-alignment limits)
        nc.vector.tensor_scalar_mul(out=ssp[:], in0=ssp[:], scalar1=S1)
        nc.vector.tensor_scalar_mul(out=ssm[:], in0=ssm[:], scalar1=S1)
        rn_m0 = sb.tile([P, 1], f32)
        rn_m1 = sb.tile([HID - P, 1], f32)
        nc.sync.dma_start(out=rn_m0[0:100, :], in_=ssp[:])
        nc.sync.dma_start(out=rn_m0[100:P, :], in_=ssm[0:28, :])
        nc.sync.dma_start(out=rn_m1[:], in_=ssm[28:100, :])

        # ======== cell table transpose: cnt_kt [128, 200] bf16 ========
        cnt_kt = []
        for kt in range(8):
            t_ = sb.tile([P, 2 * 100], bf16, tag=f"cnt_{kt}")
            for (ci, (src, co)) in enumerate(((p_bf, 0), (me_sb, 100))):
                tp = h1m1p[:, ((2 * kt + ci) % 2) * 512:((2 * kt + ci) % 2) * 512 + 100]
                nc.tensor.transpose(
                    out=tp, in_=src[:, kt * P:(kt + 1) * P], identity=ident[:])
                if (kt + ci) % 2 == 0:
                    nc.vector.tensor_copy(out=t_[:, co:co + 100], in_=tp)
                else:
                    nc.scalar.activation(out=t_[:, co:co + 100], in_=tp,
                                         func=AF.Copy)
            cnt_kt.append(t_)

        # ======== A (states x hid) -> a8 fp8 DR lhsT ========
        t1_scr = sb.tile([P, HID], bf16)
        for (mt, msl, mm, rn) in ((0, slice(0, P), P, rn_m0),
                                  (1, slice(P, HID), HID - P, rn_m1)):
            aps = (h2m0 if mt == 0 else h2m1)[0:mm, 0:HID]
            for kt in range(8):
                nc.tensor.matmul(
                    out=aps,
                    lhsT=cnt_kt[kt][:, msl],
                    rhs=wf1c_t[kt // 4][:].rearrange("p (k m) -> p k m", k=4)[:, kt % 4],
                    start=(kt == 0), stop=(kt == 7))
            nc.vector.tensor_scalar_mul(out=t1_scr[0:mm, :], in0=aps, scalar1=rn[:])
            nc.vector.tensor_tensor(
                out=a8[0:mm, mt * 208:mt * 208 + HID], in0=t1_scr[0:mm, :],
                in1=bf1b[0:mm, :], op=ALU.add)

        # ======== Bd (drugs x hid) -> bd8 fp8 DR lhsT ========
        rdS = sb.tile([P, 2], f32)
        nc.vector.tensor_scalar_mul(out=rdS[:], in0=rd2[:], scalar1=S1)
        for mt in range(2):
            bps = (h2m0 if mt == 0 else h2m1)[:, 0:HID]
            nc.tensor.matmul(out=bps, lhsT=deT_sb[:, mt * P:(mt + 1) * P],
                             rhs=wf1d_sb[:], start=True, stop=True)
            nc.scalar.activation(out=bd8[:, mt * 208:mt * 208 + HID], in_=bps,
                                 func=AF.Copy, scale=rdS[:, mt:mt + 1])

        # ======== chunk pipeline (pair-fused: 2 chunks per emit) ========
        def emit_dma_pair(c0, bb):
            # issue the cs/dm broadcast loads for pair (c0, c0+2) into buffer bb
            n0 = c0 * 512
            for (dst, rowd, eng) in ((bc_t[bb], cs_rowd, nc.sync),
                                     (bd_t[bb], dm_rowd, nc.gpsimd)):
                for hh in range(2):
                    eng.dma_start(
                        out=dst[:, hh * 512:(hh + 1) * 512],
                        in_=bass.AP(tensor=rowd.ap().tensor,
                                    offset=n0 + hh * 1024,
                                    ap=[[0, P], [1, 512]]))

        def emit_oh(pp, bb):
            # one-hot layout [p, (t, half, j)]: plane t built with one
            # [128, 1024] tensor-scalar over both halves
            nc.vector.tensor_scalar(out=sc8[pp][:, 0:1024], in0=bc_t[bb][:],
                                    scalar1=icol[:], scalar2=None,
                                    op0=ALU.is_equal)
            nc.vector.tensor_scalar(out=sc8[pp][:, 1024:2048], in0=bc_t[bb][:],
                                    scalar1=128.0, scalar2=icol[:],
                                    op0=ALU.subtract, op1=ALU.is_equal)
            nc.vector.tensor_scalar(out=sd8[pp][:, 0:1024], in0=bd_t[bb][:],
                                    scalar1=icol[:], scalar2=None,
                                    op0=ALU.is_equal)
            nc.vector.tensor_scalar(out=sd8[pp][:, 1024:2048], in0=bd_t[bb][:],
                                    scalar1=128.0, scalar2=icol[:],
                                    op0=ALU.subtract, op1=ALU.is_equal)

        def emit_h1(pp):
            a8_v = a8[:].rearrange("p (t m) -> p t m", t=2)
            bd8_v = bd8[:].rearrange("p (t m) -> p t m", t=2)
            for hh in range(2):
                sc_v = sc8[pp][:].rearrange("p (t h n) -> p t h n", t=2, h=2)[:, :, hh, :]
                sd_v = sd8[pp][:].rearrange("p (t h n) -> p t h n", t=2, h=2)[:, :, hh, :]
                hsl = slice(hh * 512, (hh + 1) * 512)
                nc.tensor.matmul(out=h1m0p[:, hsl], lhsT=a8_v[:, :, 0:P],
                                 rhs=sc_v, start=True, stop=False, perf_mode=DR)
                nc.tensor.matmul(out=h1m0p[:, hsl], lhsT=bd8_v[:, :, 0:P],
                                 rhs=sd_v, start=False, stop=True, perf_mode=DR)
                nc.tensor.matmul(out=h1m1p[0:HID - P, hsl],
                                 lhsT=a8_v[:, :, P:HID],
                                 rhs=sc_v, start=True, stop=False, perf_mode=DR)
                nc.tensor.matmul(out=h1m1p[0:HID - P, hsl],
                                 lhsT=bd8_v[:, :, P:HID],
                                 rhs=sd_v, start=False, stop=True, perf_mode=DR)
            h18_v = h18[pp][:].rearrange("p (h t n) -> p h t n", h=2, t=2)
            nc.scalar.activation(out=h18_v[:, :, 0, :], in_=h1m0p[:],
                                 func=AF.Relu, scale=S_H / S1)
            nc.vector.tensor_scalar(out=h18_v[0:HID - P, :, 1, :],
                                    in0=h1m1p[0:HID - P, :],
                                    scalar1=S_H / S1, scalar2=0.0,
                                    op0=ALU.mult, op1=ALU.max)

        def emit_h2(pp):
            w2_v = wf28_sb[:].rearrange("p (t m) -> p t m", t=2)
            h28_v = h28[pp][:].rearrange("p (h t n) -> p h t n", h=2, t=2)
            for hh in range(2):
                h1_v = h18[pp][:].rearrange("p (h t n) -> p h t n", h=2, t=2)[:, hh]
                nc.tensor.matmul(out=h2m0[:], lhsT=w2_v[:, :, 0:P], rhs=h1_v,
                                 start=True, stop=True, perf_mode=DR)
                nc.tensor.matmul(out=h2m1[0:HID - P, :], lhsT=w2_v[:, :, P:HID],
                                 rhs=h1_v, start=True, stop=True, perf_mode=DR)
                nc.scalar.activation(out=h28_v[:, hh, 0, :], in_=h2m0[:],
                                     func=AF.Relu, scale=S_H2 / (S_H * S_W2))
                nc.scalar.activation(out=h28_v[0:HID - P, hh, 1, :],
                                     in_=h2m1[0:HID - P, :],
                                     func=AF.Relu, scale=S_H2 / (S_H * S_W2))

        def emit_fwd(pp):
            # doses to fmu2[0:8, half]; base broadcast x10 into the mu region
            # rows 32:42 (start=True) where the L8 cumsum later accumulates
            w3_v = wf38_sb[:].rearrange("p (t m) -> p t m", t=2)
            for hh in range(2):
                h2_v = h28[pp][:].rearrange("p (h t n) -> p h t n", h=2, t=2)[:, hh]
                hsl = slice(hh * 512, (hh + 1) * 512)
                nc.tensor.matmul(out=fmu2[0:NDOSES - 1, hsl],
                                 lhsT=w3_v[:, :, 0:NDOSES - 1], rhs=h2_v,
                                 start=True, stop=True, perf_mode=DR)
                nc.tensor.matmul(out=fmu2[32:42, hsl],
                                 lhsT=w3_v[:, 0, 8:18],
                                 rhs=h2_v[:, 0, :], start=True, stop=False,
                                 skip_group_check=True)
                nc.tensor.matmul(out=fmu2[32:42, hsl],
                                 lhsT=w3_v[:, 1, 8:18],
                                 rhs=h2_v[:, 1, :], start=False, stop=False,
                                 skip_group_check=True)

        def emit_tail(pp, c0):
            n0 = c0 * 512
            # gb8 = softplus(d)-ln2 = ln(0.5 exp + 0.5)  (bf16)
            nc.scalar.activation(out=spf[pp][:], in_=fmu2[0:NDOSES - 1, :],
                                 func=AF.Exp, scale=1.0 / (S_H2 * S_W3))
            nc.scalar.activation(out=gb8[pp][:], in_=spf[pp][:],
                                 func=AF.Ln, scale=0.5, bias=half_col[:])
            # mu = base-bcast (already in psum, scaled S_H2*S_W3) + cumsum:
            # L8 lhsT is pre-scaled by S_H2*S_W3 so one final scale works
            for hh in range(2):
                hsl = slice(hh * 512, (hh + 1) * 512)
                nc.tensor.matmul(out=fmu2[32:32 + NDOSES, hsl], lhsT=L8S[:],
                                 rhs=gb8[pp][:, hsl], start=False, stop=True,
                                 skip_group_check=True)
            nc.vector.tensor_scalar(
                out=bass.AP(tensor=mu_sb[:].tensor, offset=n0,
                            ap=[[BS, NDOSES], [1024, 2], [1, 512]]),
                in0=fmu2[32:32 + NDOSES, :],
                scalar1=rs_col[:], scalar2=ramp[:], op0=ALU.mult, op1=ALU.add)

        pairs = [0, 4, 8, 12, 1, 5, 9, 13]
        for i, c0 in enumerate(pairs[:4]):
            emit_dma_pair(c0, i % 4)
        prev = None
        for i, c0 in enumerate(pairs):
            if i == 4:
                if prev is not None:
                    emit_tail(prev[0], prev[1])
                    prev = None
                emit_piece_gather(1)
                emit_piece_reduce(1)
                for j, cc in enumerate(pairs[4:]):
                    emit_dma_pair(cc, j % 4)
            emit_oh(i % 2, i % 4)
            emit_h1(i % 2)
            if prev is not None:
                emit_tail(prev[0], prev[1])
            emit_h2(i % 2)
            emit_fwd(i % 2)
            prev = (i % 2, c0)
        emit_tail(prev[0], prev[1])

        nc.sync.dma_start(out=mu_s[:], in_=mu_sb[:])

    return _split_sync_waits(nc) if split_waits else nc


def _get_nc():
    if "nc" not in _NC_CACHE:
        _NC_CACHE["nc"] = build_nc()
    return _NC_CACHE["nc"]


def _wrap16(vals):
    # vals [8192] in sample order k (g = k>>10, j = k&1023)
    # -> [128, 64] at [16g + (j & 15), j >> 4]
    v = vals.reshape(NG, GS // 16, 16)        # [g, j_hi, j_lo]
    v = np.transpose(v, (0, 2, 1))            # [g, j_lo, j_hi]
    return np.ascontiguousarray(v.reshape(P, GS // 16))


def make_in_maps(inputs):
    idx = np.asarray(inputs["idx"], np.int64)
    tidx = np.asarray(inputs["tidx"], np.int64)
    cf = np.asarray(inputs["cell_features"], np.float32)
    me = np.asarray(inputs["missing_emb"], np.float32)
    de = np.asarray(inputs["drug_emb"], np.float32)
    W1 = np.asarray(inputs["W1"], np.float32)
    b1 = np.asarray(inputs["b1"], np.float32)
    Wf1 = np.asarray(inputs["Wf1"], np.float32)
    bf1 = np.asarray(inputs["bf1"], np.float32)
    Wf2 = np.asarray(inputs["Wf2"], np.float32)
    bf2 = np.asarray(inputs["bf2"], np.float32)
    Wf3 = np.asarray(inputs["Wf3"], np.float32)
    bf3 = np.asarray(inputs["bf3"], np.float32)

    cs_full = (np.asarray(inputs["cell_map"], np.int64)
               + 100 * np.asarray(inputs["is_missing"], np.int64))

    # Wf2/Wf3 fp8 DR lhsT with the bias folded into pad row (t=1, p=96);
    # Wf3 columns permuted to [dose1..dose8, dose0(base)]
    perm = [1, 2, 3, 4, 5, 6, 7, 8, 0]
    Wf3p = Wf3[:, perm]
    bf3p = bf3[perm]
    wf28 = np.zeros((P, 2, 208), NP_FP8)
    wf28[:, 0, :HID] = (Wf2[0:P, :] * S_W2).astype(NP_FP8)
    wf28[:HID - P, 1, :HID] = (Wf2[P:HID, :] * S_W2).astype(NP_FP8)
    wf28[96, 1, :HID] = (bf2 * S_W2).astype(NP_FP8)
    Wf3d = np.concatenate([Wf3p[:, :8]] + [Wf3p[:, 8:9]] * 10, axis=1)
    bf3d = np.concatenate([bf3p[:8]] + [bf3p[8:9]] * 10)
    wf38 = np.zeros((P, 2, 32), NP_FP8)
    wf38[:, 0, :18] = (Wf3d[0:P, :] * S_W3).astype(NP_FP8)
    wf38[:HID - P, 1, :18] = (Wf3d[P:HID, :] * S_W3).astype(NP_FP8)
    wf38[96, 1, :18] = (bf3d * S_W3).astype(NP_FP8)

    shared = dict(
        cs8=np.ascontiguousarray(cs_full.astype(np.uint8)),
        dm8=np.ascontiguousarray(np.asarray(inputs["drug_map"]).astype(np.uint8)),
        w18=np.ascontiguousarray((W1 * S_W1).astype(NP_FP8)),
        cf8=np.ascontiguousarray((cf[:100, :].T * S_CF).astype(NP_FP8)),
        b1S=np.ascontiguousarray((b1 * SP100).astype(NP_BF16)),
        wf1c=np.ascontiguousarray(Wf1[:CEMB, :].astype(NP_BF16)),
        bf1S1=np.ascontiguousarray((bf1 * S1).astype(NP_BF16)),
        me_in=np.ascontiguousarray(me),
        de_bf=np.ascontiguousarray(de.astype(NP_BF16)),
        deT_bf=np.ascontiguousarray(de.T.astype(NP_BF16)),
        wf1d=np.ascontiguousarray(Wf1[CEMB:, :].astype(NP_BF16)),
        wf28=np.ascontiguousarray(wf28.reshape(P, 2 * 208)),
        wf38=np.ascontiguousarray(wf38.reshape(P, 2 * 32)),
    )

    in_maps = []
    for c in range(NCORES):
        ic = idx[c * BS:(c + 1) * BS]
        tc_ = tidx[c * BS:(c + 1) * BS]
        m = dict(shared)
        m["u_idx"] = _wrap16((ic & (SLAB - 1)).astype(np.uint16))
        m["u_tidx"] = _wrap16((tc_ & (SLAB - 1)).astype(np.uint16))
        m["qrow_cs"] = np.ascontiguousarray((ic >> 14).astype(NP_BF16))
        m["qrow_dm"] = np.ascontiguousarray((tc_ >> 14).astype(NP_BF16))
        in_maps.append(m)
    return in_maps


def kernel(**inputs):
    nc = _get_nc()
    in_maps = make_in_maps(inputs)
    last_err = None
    for _attempt in range(3):
        try:
            res = run_bass_kernel_spmd(nc, in_maps, core_ids=list(range(NCORES)))
            return np.ascontiguousarray(np.concatenate(
                [res.results[c]["mu_s"].T for c in range(NCORES)], axis=0))
        except Exception as e:  # wedged device sometimes recovers on retry
            last_err = e
    raise last_err


# revision 32
# speedup vs baseline: 1.2673x; 1.0280x over previous
"""Trainium2 Bass kernel for nn_DrugResponsePrior (embedding_lookup).

Spec guarantees: cell_map < 100, is_missing in {0,1}, drug_map < 256.  So each
row depends only on cs = cell_map[idx]+100*is_missing[idx] (200 states) and
dm = drug_map[tidx] (256 drugs).

Fully data-parallel (8 cores x 8192 samples, no collectives), fp8 pipeline:
  1. cs/dm u8 tables in a 16-slab SBUF layout; per-sample lookup with gpsimd
     indirect_copy + one-hot slab mask + group-reduce matmul (bf16).
  2. Tables A = l2n(cell table) @ Wf1c + bf1 (200x200) and Bd = l2n(drug_emb)
     @ Wf1d (256x200) built on device, stored as scaled fp8 DoubleRow lhsT.
  3. Per 512-sample chunk: one-hot matrices (fp8) over cs/dm, h1/h2/fwd as
     fp8 DoubleRow matmuls (0.5 cyc/row), biases folded into pad rows or
     activation bias, softplus residual + cumsum via one bf16 9x9 matmul plus
     an exact f32 ln2-ramp add.
Output mu is written [9, BS] per core; host transposes/concats.
Numerics validated on host: rel_fro ~ 5e-4 (gate 2e-2) with these scales.
"""
import sys

if "/opt/trn_rl_repo" not in sys.path:
    sys.path.insert(0, "/opt/trn_rl_repo")

import numpy as np
import ml_dtypes

import concourse.bass as bass
import concourse.mybir as mybir
import concourse.tile as tile
from concourse.bass_utils import run_bass_kernel_spmd

f32 = mybir.dt.float32
bf16 = mybir.dt.bfloat16
fp8 = mybir.dt.float8e4
u16 = mybir.dt.uint16
u8 = mybir.dt.uint8

NP_BF16 = ml_dtypes.bfloat16
NP_FP8 = ml_dtypes.float8_e4m3

B = 65536
R = 262144
NDRUG = 256
NFEAT = 1024
CEMB = 1024
DEMB = 128
HID = 200
NDOSES = 9
NCORES = 8

BS = B // NCORES            # 8192 samples per core
P = 128
NG = 8                      # index groups (16 partitions each)
GS = BS // NG               # 1024 samples per group
SLAB = R // 16              # 16384 entries per slab partition
NCHUNK = BS // 512          # 16 chunks of 512 samples
EPS = 1e-12
LN2 = float(np.log(2.0))

# power-of-two scales for fp8 operands (validated on host: rel_fro ~ 5e-4)
S_CF = 8.0                  # cell_features
S_W1 = 64.0                 # W1
SP100 = S_CF * S_W1         # psum scale of cf @ W1
S1 = 256.0                  # A / Bd tables (h1 psum scale)
S_H = 64.0                  # h1 activations
S_W2 = 256.0                # Wf2
S_H2 = 64.0                 # h2 activations
S_W3 = 1024.0               # Wf3

_NC_CACHE = {}


def _split_sync_waits(nc, limit=1):
    """walrus accepts at most one sync-wait per instruction; hoist excess
    waits onto same-engine NoOps inserted just before."""
    ctr = 0
    for bb in nc.main_func.blocks:
        new_list = []
        for inst in bb.instructions:
            si = inst.sync_info
            if si is not None and si.on_wait and len(si.on_wait) > limit:
                waits = list(si.on_wait)
                head, tail = waits[:-limit], waits[-limit:]
                for j in range(0, len(head), limit):
                    nop = mybir.InstNoOp(name=f"waitnop-{ctr}", engine=inst.engine)
                    ctr += 1
                    nop.sync_info = mybir.SyncInfo(
                        on_wait=list(head[j : j + limit]), on_update=[]
                    )
                    new_list.append(nop)
                inst.sync_info = mybir.SyncInfo(
                    on_wait=list(tail),
                    on_update=list(si.on_update) if si.on_update else [],
                )
            new_list.append(inst)
        bb.instructions[:] = new_list
    return nc


def build_nc(split_waits=True):
    nc = bass.Bass(num_devices=NCORES)
    AF = mybir.ActivationFunctionType
    ALU = mybir.AluOpType
    DR = mybir.MatmulPerfMode.DoubleRow

    # ---------------- kernel I/O ----------------
    cs8 = nc.dram_tensor("cs8", [R], u8, kind="ExternalInput")
    dm8 = nc.dram_tensor("dm8", [R], u8, kind="ExternalInput")
    u_idx = nc.dram_tensor("u_idx", [P, GS // 16], u16, kind="ExternalInput")
    u_tidx = nc.dram_tensor("u_tidx", [P, GS // 16], u16, kind="ExternalInput")
    qrow_cs = nc.dram_tensor("qrow_cs", [NG * GS], bf16, kind="ExternalInput")
    qrow_dm = nc.dram_tensor("qrow_dm", [NG * GS], bf16, kind="ExternalInput")
    w18 = nc.dram_tensor("w18", [NFEAT, CEMB], fp8, kind="ExternalInput")
    cf8 = nc.dram_tensor("cf8", [NFEAT, 100], fp8, kind="ExternalInput")
    b1S = nc.dram_tensor("b1S", [CEMB], bf16, kind="ExternalInput")
    wf1c = nc.dram_tensor("wf1c", [CEMB, HID], bf16, kind="ExternalInput")
    bf1S1 = nc.dram_tensor("bf1S1", [HID], bf16, kind="ExternalInput")
    me_in = nc.dram_tensor("me_in", [100, CEMB], f32, kind="ExternalInput")
    de_bf = nc.dram_tensor("de_bf", [NDRUG, DEMB], bf16, kind="ExternalInput")
    deT_bf = nc.dram_tensor("deT_bf", [DEMB, NDRUG], bf16, kind="ExternalInput")
    wf1d = nc.dram_tensor("wf1d", [DEMB, HID], bf16, kind="ExternalInput")
    wf28 = nc.dram_tensor("wf28", [P, 2 * 208], fp8, kind="ExternalInput")
    wf38 = nc.dram_tensor("wf38", [P, 2 * 32], fp8, kind="ExternalInput")
    mu_s = nc.dram_tensor("mu_s", [NDOSES, BS], f32, kind="ExternalOutput")

    # internal DRAM rows for the per-sample cs/dm values (broadcast reload)
    cs_rowd = nc.dram_tensor("cs_rowd", [BS], bf16)
    dm_rowd = nc.dram_tensor("dm_rowd", [BS], bf16)

    # inline constants
    icol_c = nc.inline_tensor(
        np.arange(P, dtype=np.float32).reshape(P, 1), name="icol_c")
    qcol_c = nc.inline_tensor(
        (np.arange(P, dtype=np.float32) % 16).reshape(P, 1), name="qcol_c")
    grp_rd_c = nc.inline_tensor(  # [128, 8] group-reduce lhsT (bf16)
        np.array([[1.0 if (k // 16) == g else 0.0 for g in range(NG)]
                  for k in range(P)], NP_BF16), name="grp_rd_c")
    ones100_c = nc.inline_tensor(np.ones((1, 100), NP_BF16), name="ones100_c")
    # L8S[k, o] = S_H2*S_W3 iff dose k+1 contributes to output o (k+1 <= o);
    # pre-scaled so it accumulates onto the (scaled) base-broadcast psum
    L8np = np.zeros((NDOSES - 1, NDOSES), NP_BF16)
    for k in range(NDOSES - 1):
        L8np[k, k + 1:] = S_H2 * S_W3
    L8S_c = nc.inline_tensor(L8np, name="L8S_c")
    ramp_c = nc.inline_tensor(
        (LN2 * np.arange(NDOSES, dtype=np.float32)).reshape(NDOSES, 1),
        name="ramp_c")
    ident_c = nc.inline_tensor(np.eye(100, dtype=np.float32), name="ident_c")


    with tile.TileContext(nc) as tc, \
            tc.tile_pool(name="sb", bufs=1) as sb, \
            tc.tile_pool(name="ps", bufs=1, space="PSUM") as ps:

        # ======== psum banks: pair-fused 2-bank tiles (8 banks total) ========
        h1m0p = ps.tile([P, 1024], f32, tag="h1m0p")    # 2 banks
        h1m1p = ps.tile([P, 1024], f32, tag="h1m1p")    # 2 banks
        h2m0 = ps.tile([P, 512], f32, tag="h2m0")
        h2m1 = ps.tile([P, 512], f32, tag="h2m1")
        # fmu2 [128, 1024]: rows 0:8 = f9 doses / piece vp, 64:66 = f9 base,
        # 32:41 = mu; cols half*512 per chunk-in-pair
        fmu2 = ps.tile([P, 1024], f32, tag="fmu2")

        # ======== small consts to SBUF ========
        icol = sb.tile([P, 1], f32)
        nc.sync.dma_start(out=icol[:], in_=icol_c[:])
        qcol = sb.tile([P, 1], f32)
        nc.sync.dma_start(out=qcol[:], in_=qcol_c[:])
        grp_rd = sb.tile([P, NG], bf16)
        nc.sync.dma_start(out=grp_rd[:], in_=grp_rd_c[:])
        ones100 = sb.tile([1, 100], bf16)
        nc.sync.dma_start(out=ones100[:], in_=ones100_c[:])
        L8S = sb.tile([NDOSES - 1, NDOSES], bf16)
        nc.sync.dma_start(out=L8S[:], in_=L8S_c[:])
        rs_col = sb.tile([NDOSES, 1], f32)
        nc.vector.memset(rs_col[:], 1.0 / (S_H2 * S_W3))
        ramp = sb.tile([NDOSES, 1], f32)
        nc.sync.dma_start(out=ramp[:], in_=ramp_c[:])
        ident = sb.tile([100, 100], f32)
        nc.sync.dma_start(out=ident[:], in_=ident_c[:])
        half_col = sb.tile([NDOSES - 1, 1], f32)
        nc.vector.memset(half_col[:], 0.5)


        # ======== PE warm-up: keep the array busy so the clock ramps ========
        warm = sb.tile([1, 512], bf16)
        nc.vector.memset(warm[:], 1.0)
        for _ in range(8):
            nc.tensor.matmul(out=h2m0[0:1, :], lhsT=warm[:, 0:1],
                             rhs=warm[:], start=True, stop=True)

        # ======== index / table DMAs ========
        u_idx_sb = sb.tile([P, GS // 16], u16)
        nc.sync.dma_start(out=u_idx_sb[:], in_=u_idx[:])
        u_tidx_sb = sb.tile([P, GS // 16], u16)
        nc.sync.dma_start(out=u_tidx_sb[:], in_=u_tidx[:])

        cs_slab = sb.tile([P, SLAB], u8)
        dm_slab = sb.tile([P, SLAB], u8)
        for (slab, tab, eng) in ((cs_slab, cs8, nc.sync), (dm_slab, dm8, nc.gpsimd)):
            for h in range(2):  # split each table load across 2 issues
                eng.dma_start(
                    out=slab[h * 64:(h + 1) * 64, :],
                    in_=bass.AP(tensor=tab.ap().tensor, offset=0,
                                ap=[[0, 4], [SLAB, 16], [1, SLAB]]))

        # q values (idx >> 14) broadcast to each 16-partition group
        qbc_cs = sb.tile([P, GS], bf16)
        qbc_dm = sb.tile([P, GS], bf16)
        for (t_, row) in ((qbc_cs, qrow_cs), (qbc_dm, qrow_dm)):
            nc.scalar.dma_start(
                out=t_[:],
                in_=bass.AP(tensor=row.ap().tensor, offset=0,
                            ap=[[GS, NG], [0, 16], [1, GS]]))

        # ======== weights to SBUF ========
        # W1 (fp8, DoubleRow rhs layout): per kt a [128, 2, 1024] view
        w1t = []
        for kt in range(4):
            t_ = sb.tile([P, 2 * CEMB], fp8, tag=f"w1t_{kt}", name=f"w1t_{kt}")
            (nc.sync if kt % 2 == 0 else nc.scalar).dma_start(
                out=t_[:],
                in_=bass.AP(tensor=w18.ap().tensor, offset=kt * 256 * CEMB,
                            ap=[[CEMB, P], [P * CEMB, 2], [1, CEMB]]))
            w1t.append(t_)
        # cfT (fp8 DR lhsT): one [128, 4*2*100] tile, [p, (kt, t, c)]
        cft = sb.tile([P, 4 * 2 * 112], fp8)
        nc.gpsimd.dma_start(
            out=cft[:].rearrange("p (kt t c) -> p kt t c", kt=4, t=2)[:, :, :, 0:100],
            in_=bass.AP(tensor=cf8.ap().tensor, offset=0,
                        ap=[[100, P], [256 * 100, 4], [P * 100, 2], [1, 100]]))
        b1row = sb.tile([1, CEMB], bf16)
        nc.scalar.dma_start(out=b1row[:], in_=b1S[:].rearrange("(one n) -> one n", one=1))
        # Wf1c bf16: two [128, 4, 200] tiles (kt-major)
        wf1c_t = []
        for h in range(2):
            t_ = sb.tile([P, 4 * HID], bf16, tag=f"wf1c_{h}", name=f"wf1c_{h}")
            nc.scalar.dma_start(
                out=t_[:],
                in_=bass.AP(tensor=wf1c.ap().tensor, offset=h * 4 * P * HID,
                            ap=[[HID, P], [P * HID, 4], [1, HID]]))
            wf1c_t.append(t_)
        bf1b = sb.tile([P, HID], bf16)
        nc.scalar.dma_start(
            out=bf1b[:],
            in_=bass.AP(tensor=bf1S1.ap().tensor, offset=0, ap=[[0, P], [1, HID]]))
        me_sb = sb.tile([100, CEMB], f32)
        nc.scalar.dma_start(out=me_sb[:], in_=me_in[:])
        de2 = sb.tile([P, 2 * DEMB], bf16)  # [p, (mt, f)]
        nc.scalar.dma_start(
            out=de2[:],
            in_=bass.AP(tensor=de_bf.ap().tensor, offset=0,
                        ap=[[DEMB, P], [P * DEMB, 2], [1, DEMB]]))
        deT_sb = sb.tile([DEMB, NDRUG], bf16)
        nc.scalar.dma_start(out=deT_sb[:], in_=deT_bf[:])
        wf1d_sb = sb.tile([DEMB, HID], bf16)
        nc.scalar.dma_start(out=wf1d_sb[:], in_=wf1d[:])
        wf28_sb = sb.tile([P, 2 * 208], fp8)
        nc.sync.dma_start(out=wf28_sb[:], in_=wf28[:])
        wf38_sb = sb.tile([P, 2 * 32], fp8)
        nc.sync.dma_start(out=wf38_sb[:], in_=wf38[:])

        # ======== static chunk tiles + pad memsets ========
        # pair tiles: layout [p, (half, t, j)] = [128, 2048]
        h18 = [sb.tile([P, 2048], fp8, tag=f"h18_{i}", name=f"h18_{i}") for i in range(2)]
        h28 = [sb.tile([P, 2048], fp8, tag=f"h28_{i}", name=f"h28_{i}") for i in range(2)]
        for i in range(2):
            for hh in range(2):
                t1c = hh * 1024 + 512
                nc.vector.memset(h18[i][64:P, t1c:t1c + 512], 0.0)
                nc.vector.memset(h18[i][96:97, t1c:t1c + 512], S_H)
                nc.gpsimd.memset(h28[i][64:P, t1c:t1c + 512], 0.0)
                nc.gpsimd.memset(h28[i][96:97, t1c:t1c + 512], S_H2)
        a8 = sb.tile([P, 2 * 208], fp8)
        nc.vector.memset(a8[64:P, 208:416], 0.0)
        bd8 = sb.tile([P, 2 * 208], fp8)
        sc8 = [sb.tile([P, 2048], fp8, tag=f"sc8_{i}", name=f"sc8_{i}") for i in range(2)]
        sd8 = [sb.tile([P, 2048], fp8, tag=f"sd8_{i}", name=f"sd8_{i}") for i in range(2)]
        bc_t = [sb.tile([P, 1024], bf16, tag=f"bc_{i}", name=f"bc_{i}") for i in range(4)]
        bd_t = [sb.tile([P, 1024], bf16, tag=f"bd_{i}", name=f"bd_{i}") for i in range(4)]
        gb8 = [sb.tile([NDOSES - 1, 1024], bf16, tag=f"gb_{i}", name=f"gb_{i}") for i in range(2)]
        spf = [sb.tile([NDOSES - 1, 1024], f32, tag=f"spf_{i}", name=f"spf_{i}") for i in range(2)]
        mu_sb = sb.tile([NDOSES, BS], f32)

        # piece tiles
        g_cs = sb.tile([P, GS], u8)
        g_dm = sb.tile([P, GS], u8)
        qm_t = [sb.tile([P, 512], bf16, tag=f"qm_{i}", name=f"qm_{i}") for i in range(2)]
        gtb_t = [sb.tile([P, 512], bf16, tag=f"gtb_{i}", name=f"gtb_{i}") for i in range(2)]
        v8_t = [sb.tile([NG, 512], bf16, tag=f"v8_{i}", name=f"v8_{i}") for i in range(2)]

        # ======== piece part 1: gathers + masks ========
        def emit_piece_gather(t):
            tsl = slice(t * 512, (t + 1) * 512)
            isl = slice(t * 32, (t + 1) * 32)
            for (k, gt, slab, ut, qbc) in (
                    (0, g_cs, cs_slab, u_idx_sb, qbc_cs),
                    (1, g_dm, dm_slab, u_tidx_sb, qbc_dm)):
                nc.gpsimd.indirect_copy(
                    out=gt[:, tsl].rearrange("p (n one) -> p n one", one=1),
                    data=slab[:], idxs=ut[:, isl],
                    i_know_ap_gather_is_preferred=True)
                nc.vector.tensor_scalar(
                    out=qm_t[k][:], in0=qbc[:, tsl], scalar1=qcol[:],
                    scalar2=None, op0=ALU.is_equal)
                nc.scalar.activation(out=gtb_t[k][:], in_=gt[:, tsl], func=AF.Copy)
                nc.vector.tensor_tensor(
                    out=gtb_t[k][:], in0=gtb_t[k][:], in1=qm_t[k][:], op=ALU.mult)

        # ======== piece part 2: group-reduce + rowd store ========
        def emit_piece_reduce(t):
            for (k, rowd, bank) in ((0, cs_rowd, 0), (1, dm_rowd, 1)):
                vp = fmu2[0:NG, bank * 512:(bank + 1) * 512]
                nc.tensor.matmul(out=vp, lhsT=grp_rd[:], rhs=gtb_t[k][:],
                                 start=True, stop=True)
                nc.scalar.activation(out=v8_t[k][:], in_=vp, func=AF.Copy)
                nc.scalar.dma_start(
                    out=bass.AP(tensor=rowd.ap().tensor, offset=t * 512,
                                ap=[[GS, NG], [1, 512]]),
                    in_=v8_t[k][:])

        emit_piece_gather(0)

        # ======== P100 = relu(cf @ W1 + b1), scaled fp8 DR ========
        p_bf = sb.tile([100, CEMB], f32)
        for nh in range(2):
            pps = h1m0p[0:100, nh * 512:(nh + 1) * 512]
            for kt in range(4):
                nc.tensor.matmul(
                    out=pps,
                    lhsT=cft[:].rearrange("p (kt t c) -> p kt t c", kt=4, t=2)[:, kt, :, 0:100],
                    rhs=w1t[kt][:].rearrange("p (t n) -> p t n", t=2)[:, :, nh * 512:(nh + 1) * 512],
                    start=(kt == 0), stop=False, perf_mode=DR)
            nc.tensor.matmul(
                out=pps, lhsT=ones100[:], rhs=b1row[:, nh * 512:(nh + 1) * 512],
                start=False, stop=True)
        nc.scalar.activation(out=p_bf[:], in_=h1m0p[0:100, :],
                             func=AF.Relu, scale=1.0 / SP100)

        emit_piece_reduce(0)

        # ======== norms (squared-sum via activation accum) ========
        sq_scr = sb.tile([100, CEMB], bf16)  # discarded; only accum_out matters
        ssp = sb.tile([100, 1], f32)
        ssm = sb.tile([100, 1], f32)
        nc.scalar.activation(out=sq_scr[:], in_=p_bf[:], func=AF.Square,
                             accum_out=ssp[:])
        nc.scalar.activation(out=sq_scr[:], in_=me_sb[:], func=AF.Square,
                             accum_out=ssm[:])
        rd2 = sb.tile([P, 2], f32)
        sqd_scr = sb.tile([P, DEMB], bf16)
        for mt in range(2):
            nc.scalar.activation(out=sqd_scr[:], in_=de2[:, mt * DEMB:(mt + 1) * DEMB],
                                 func=AF.Square, accum_out=rd2[:, mt:mt + 1])
        for ss in (ssp, ssm):
            nc.scalar.activation(out=ss[:], in_=ss[:], func=AF.Sqrt)
            nc.vector.tensor_scalar_max(out=ss[:], in0=ss[:], scalar1=EPS)
            nc.vector.reciprocal(out=ss[:], in_=ss[:])
        nc.scalar.activation(out=rd2[:], in_=rd2[:], func=AF.Sqrt)
        nc.vector.tensor_scalar_max(out=rd2[:], in0=rd2[:], scalar1=EPS)
        nc.vector.reciprocal(out=rd2[:], in_=rd2[:])
        # rnS[mt] = S1 * rnorm for A m-tiles (states on partitions);
        # assembled with sbuf-to-sbuf DMAs (no partition-alignment limits)
        nc.vector.tensor_scalar_mul(out=ssp[:], in0=ssp[:], scalar1=S1)
        nc.vector.tensor_scalar_mul(out=ssm[:], in0=ssm[:], scalar1=S1)
        rn_m0 = sb.tile([P, 1], f32)
        rn_m1 = sb.tile([HID - P, 1], f32)
        nc.sync.dma_start(out=rn_m0[0:100, :], in_=ssp[:])
        nc.sync.dma_start(out=rn_m0[100:P, :], in_=ssm[0:28, :])
        nc.sync.dma_start(out=rn_m1[:], in_=ssm[28:100, :])

        # ======== cell table transpose: cnt_kt [128, 200] bf16 ========
        cnt_kt = []
        for kt in range(8):
            t_ = sb.tile([P, 2 * 100], bf16, tag=f"cnt_{kt}")
            for (ci, (src, co)) in enumerate(((p_bf, 0), (me_sb, 100))):
                tp = h1m1p[:, ((2 * kt + ci) % 2) * 512:((2 * kt + ci) % 2) * 512 + 100]
                nc.tensor.transpose(
                    out=tp, in_=src[:, kt * P:(kt + 1) * P], identity=ident[:])
                if (kt + ci) % 2 == 0:
                    nc.vector.tensor_copy(out=t_[:, co:co + 100], in_=tp)
                else:
                    nc.scalar.activation(out=t_[:, co:co + 100], in_=tp,
                                         func=AF.Copy)
            cnt_kt.append(t_)

        # ======== A (states x hid) -> a8 fp8 DR lhsT ========
        t1_scr = sb.tile([P, HID], bf16)
        for (mt, msl, mm, rn) in ((0, slice(0, P), P, rn_m0),
                                  (1, slice(P, HID), HID - P, rn_m1)):
            aps = (h2m0 if mt == 0 else h2m1)[0:mm, 0:HID]
            for kt in range(8):
                nc.tensor.matmul(
                    out=aps,
                    lhsT=cnt_kt[kt][:, msl],
                    rhs=wf1c_t[kt // 4][:].rearrange("p (k m) -> p k m", k=4)[:, kt % 4],
                    start=(kt == 0), stop=(kt == 7))
            nc.vector.tensor_scalar_mul(out=t1_scr[0:mm, :], in0=aps, scalar1=rn[:])
            nc.vector.tensor_tensor(
                out=a8[0:mm, mt * 208:mt * 208 + HID], in0=t1_scr[0:mm, :],
                in1=bf1b[0:mm, :], op=ALU.add)

        # ======== Bd (drugs x hid) -> bd8 fp8 DR lhsT ========
        rdS = sb.tile([P, 2], f32)
        nc.vector.tensor_scalar_mul(out=rdS[:], in0=rd2[:], scalar1=S1)
        for mt in range(2):
            bps = (h2m0 if mt == 0 else h2m1)[:, 0:HID]
            nc.tensor.matmul(out=bps, lhsT=deT_sb[:, mt * P:(mt + 1) * P],
                             rhs=wf1d_sb[:], start=True, stop=True)
            nc.scalar.activation(out=bd8[:, mt * 208:mt * 208 + HID], in_=bps,
                                 func=AF.Copy, scale=rdS[:, mt:mt + 1])

        # ======== chunk pipeline (pair-fused: 2 chunks per emit) ========
        def emit_dma_pair(c0, bb):
            # issue the cs/dm broadcast loads for pair (c0, c0+2) into buffer bb
            n0 = c0 * 512
            for (dst, rowd, eng) in ((bc_t[bb], cs_rowd, nc.sync),
                                     (bd_t[bb], dm_rowd, nc.gpsimd)):
                for hh in range(2):
                    eng.dma_start(
                        out=dst[:, hh * 512:(hh + 1) * 512],
                        in_=bass.AP(tensor=rowd.ap().tensor,
                                    offset=n0 + hh * 1024,
                                    ap=[[0, P], [1, 512]]))

        def emit_oh(pp, bb):
            # one-hot layout [p, (t, half, j)]: plane t built with one
            # [128, 1024] tensor-scalar over both halves
            nc.vector.tensor_scalar(out=sc8[pp][:, 0:1024], in0=bc_t[bb][:],
                                    scalar1=icol[:], scalar2=None,
                                    op0=ALU.is_equal)
            nc.vector.tensor_scalar(out=sc8[pp][:, 1024:2048], in0=bc_t[bb][:],
                                    scalar1=128.0, scalar2=icol[:],
                                    op0=ALU.subtract, op1=ALU.is_equal)
            nc.vector.tensor_scalar(out=sd8[pp][:, 0:1024], in0=bd_t[bb][:],
                                    scalar1=icol[:], scalar2=None,
                                    op0=ALU.is_equal)
            nc.vector.tensor_scalar(out=sd8[pp][:, 1024:2048], in0=bd_t[bb][:],
                                    scalar1=128.0, scalar2=icol[:],
                                    op0=ALU.subtract, op1=ALU.is_equal)

        def emit_h1(pp):
            a8_v = a8[:].rearrange("p (t m) -> p t m", t=2)
            bd8_v = bd8[:].rearrange("p (t m) -> p t m", t=2)
            for hh in range(2):
                sc_v = sc8[pp][:].rearrange("p (t h n) -> p t h n", t=2, h=2)[:, :, hh, :]
                sd_v = sd8[pp][:].rearrange("p (t h n) -> p t h n", t=2, h=2)[:, :, hh, :]
                hsl = slice(hh * 512, (hh + 1) * 512)
                nc.tensor.matmul(out=h1m0p[:, hsl], lhsT=a8_v[:, :, 0:P],
                                 rhs=sc_v, start=True, stop=False, perf_mode=DR)
                nc.tensor.matmul(out=h1m0p[:, hsl], lhsT=bd8_v[:, :, 0:P],
                                 rhs=sd_v, start=False, stop=True, perf_mode=DR)
                nc.tensor.matmul(out=h1m1p[0:HID - P, hsl],
                                 lhsT=a8_v[:, :, P:HID],
                                 rhs=sc_v, start=True, stop=False, perf_mode=DR)
                nc.tensor.matmul(out=h1m1p[0:HID - P, hsl],
                                 lhsT=bd8_v[:, :, P:HID],
                                 rhs=sd_v, start=False, stop=True, perf_mode=DR)
            h18_v = h18[pp][:].rearrange("p (h t n) -> p h t n", h=2, t=2)
            nc.scalar.activation(out=h18_v[:, :, 0, :], in_=h1m0p[:],
                                 func=AF.Relu, scale=S_H / S1)
            nc.vector.tensor_scalar(out=h18_v[0:HID - P, :, 1, :],
                                    in0=h1m1p[0:HID - P, :],
                                    scalar1=S_H / S1, scalar2=0.0,
                                    op0=ALU.mult, op1=ALU.max)

        def emit_h2(pp):
            w2_v = wf28_sb[:].rearrange("p (t m) -> p t m", t=2)
            h28_v = h28[pp][:].rearrange("p (h t n) -> p h t n", h=2, t=2)
            for hh in range(2):
                h1_v = h18[pp][:].rearrange("p (h t n) -> p h t n", h=2, t=2)[:, hh]
                nc.tensor.matmul(out=h2m0[:], lhsT=w2_v[:, :, 0:P], rhs=h1_v,
                                 start=True, stop=True, perf_mode=DR)
                nc.tensor.matmul(out=h2m1[0:HID - P, :], lhsT=w2_v[:, :, P:HID],
                                 rhs=h1_v, start=True, stop=True, perf_mode=DR)
                nc.scalar.activation(out=h28_v[:, hh, 0, :], in_=h2m0[:],
                                     func=AF.Relu, scale=S_H2 / (S_H * S_W2))
                nc.vector.tensor_scalar(out=h28_v[0:HID - P, hh, 1, :],
                                        in0=h2m1[0:HID - P, :],
                                        scalar1=S_H2 / (S_H * S_W2), scalar2=0.0,
                                        op0=ALU.mult, op1=ALU.max)

        def emit_fwd(pp):
            # doses to fmu2[0:8, half]; base broadcast x10 into the mu region
            # rows 32:42 (start=True) where the L8 cumsum later accumulates
            w3_v = wf38_sb[:].rearrange("p (t m) -> p t m", t=2)
            for hh in range(2):
                h2_v = h28[pp][:].rearrange("p (h t n) -> p h t n", h=2, t=2)[:, hh]
                hsl = slice(hh * 512, (hh + 1) * 512)
                nc.tensor.matmul(out=fmu2[0:NDOSES - 1, hsl],
                                 lhsT=w3_v[:, :, 0:NDOSES - 1], rhs=h2_v,
                                 start=True, stop=True, perf_mode=DR)
                nc.tensor.matmul(out=fmu2[32:42, hsl],
                                 lhsT=w3_v[:, 0, 8:18],
                                 rhs=h2_v[:, 0, :], start=True, stop=False,
                                 skip_group_check=True)
                nc.tensor.matmul(out=fmu2[32:42, hsl],
                                 lhsT=w3_v[:, 1, 8:18],
                                 rhs=h2_v[:, 1, :], start=False, stop=False,
                                 skip_group_check=True)

        def emit_expln(pp):
            # gb8 = softplus(d)-ln2 = ln(0.5 exp + 0.5)  (bf16)
            nc.scalar.activation(out=spf[pp][:], in_=fmu2[0:NDOSES - 1, :],
                                 func=AF.Exp, scale=1.0 / (S_H2 * S_W3))
            nc.scalar.activation(out=gb8[pp][:], in_=spf[pp][:],
                                 func=AF.Ln, scale=0.5, bias=half_col[:])

        def emit_mu(pp, c0):
            n0 = c0 * 512
            # mu = base-bcast (already in psum, scaled S_H2*S_W3) + cumsum:
            # L8 lhsT is pre-scaled by S_H2*S_W3 so one final scale works
            for hh in range(2):
                hsl = slice(hh * 512, (hh + 1) * 512)
                nc.tensor.matmul(out=fmu2[32:32 + NDOSES, hsl], lhsT=L8S[:],
                                 rhs=gb8[pp][:, hsl], start=False, stop=True,
                                 skip_group_check=True)
            nc.scalar.activation(
                out=bass.AP(tensor=mu_sb[:].tensor, offset=n0,
                            ap=[[BS, NDOSES], [1024, 2], [1, 512]]),
                in_=fmu2[32:32 + NDOSES, :],
                func=AF.Identity, scale=1.0 / (S_H2 * S_W3), bias=ramp[:])

        pairs = [0, 4, 8, 12, 1, 5, 9, 13]
        for i, c0 in enumerate(pairs[:4]):
            emit_dma_pair(c0, i % 4)
        prev = None
        for i, c0 in enumerate(pairs):
            if i == 4:
                if prev is not None:
                    emit_mu(prev[0], prev[1])
                    prev = None
                emit_piece_gather(1)
                emit_piece_reduce(1)
                for j, cc in enumerate(pairs[4:]):
                    emit_dma_pair(cc, j % 4)
            emit_oh(i % 2, i % 4)
            emit_h1(i % 2)
            if prev is not None:
                emit_mu(prev[0], prev[1])
            emit_h2(i % 2)
            emit_fwd(i % 2)
            emit_expln(i % 2)
            prev = (i % 2, c0)
        emit_mu(prev[0], prev[1])

        nc.sync.dma_start(out=mu_s[:], in_=mu_sb[:])

    return _split_sync_waits(nc) if split_waits else nc


def _get_nc():
    if "nc" not in _NC_CACHE:
        _NC_CACHE["nc"] = build_nc()
    return _NC_CACHE["nc"]


def _wrap16(vals):
    # vals [8192] in sample order k (g = k>>10, j = k&1023)
    # -> [128, 64] at [16g + (j & 15), j >> 4]
    v = vals.reshape(NG, GS // 16, 16)        # [g, j_hi, j_lo]
    v = np.transpose(v, (0, 2, 1))            # [g, j_lo, j_hi]
    return np.ascontiguousarray(v.reshape(P, GS // 16))


def make_in_maps(inputs):
    idx = np.asarray(inputs["idx"], np.int64)
    tidx = np.asarray(inputs["tidx"], np.int64)
    cf = np.asarray(inputs["cell_features"], np.float32)
    me = np.asarray(inputs["missing_emb"], np.float32)
    de = np.asarray(inputs["drug_emb"], np.float32)
    W1 = np.asarray(inputs["W1"], np.float32)
    b1 = np.asarray(inputs["b1"], np.float32)
    Wf1 = np.asarray(inputs["Wf1"], np.float32)
    bf1 = np.asarray(inputs["bf1"], np.float32)
    Wf2 = np.asarray(inputs["Wf2"], np.float32)
    bf2 = np.asarray(inputs["bf2"], np.float32)
    Wf3 = np.asarray(inputs["Wf3"], np.float32)
    bf3 = np.asarray(inputs["bf3"], np.float32)

    cs_full = (np.asarray(inputs["cell_map"], np.int64)
               + 100 * np.asarray(inputs["is_missing"], np.int64))

    # Wf2/Wf3 fp8 DR lhsT with the bias folded into pad row (t=1, p=96);
    # Wf3 columns permuted to [dose1..dose8, dose0(base)]
    perm = [1, 2, 3, 4, 5, 6, 7, 8, 0]
    Wf3p = Wf3[:, perm]
    bf3p = bf3[perm]
    wf28 = np.zeros((P, 2, 208), NP_FP8)
    wf28[:, 0, :HID] = (Wf2[0:P, :] * S_W2).astype(NP_FP8)
    wf28[:HID - P, 1, :HID] = (Wf2[P:HID, :] * S_W2).astype(NP_FP8)
    wf28[96, 1, :HID] = (bf2 * S_W2).astype(NP_FP8)
    Wf3d = np.concatenate([Wf3p[:, :8]] + [Wf3p[:, 8:9]] * 10, axis=1)
    bf3d = np.concatenate([bf3p[:8]] + [bf3p[8:9]] * 10)
    wf38 = np.zeros((P, 2, 32), NP_FP8)
    wf38[:, 0, :18] = (Wf3d[0:P, :] * S_W3).astype(NP_FP8)
    wf38[:HID - P, 1, :18] = (Wf3d[P:HID, :] * S_W3).astype(NP_FP8)
    wf38[96, 1, :18] = (bf3d * S_W3).astype(NP_FP8)

    shared = dict(
        cs8=np.ascontiguousarray(cs_full.astype(np.uint8)),
        dm8=np.ascontiguousarray(np.asarray(inputs["drug_map"]).astype(np.uint8)),
        w18=np.ascontiguousarray((W1 * S_W1).astype(NP_FP8)),
        cf8=np.ascontiguousarray((cf[:100, :].T * S_CF).astype(NP_FP8)),
        b1S=np.ascontiguousarray((b1 * SP100).astype(NP_BF16)),
        wf1c=np.ascontiguousarray(Wf1[:CEMB, :].astype(NP_BF16)),
        bf1S1=np.ascontiguousarray((bf1 * S1).astype(NP_BF16)),
        me_in=np.ascontiguousarray(me),
        de_bf=np.ascontiguousarray(de.astype(NP_BF16)),
        deT_bf=np.ascontiguousarray(de.T.astype(NP_BF16)),
        wf1d=np.ascontiguousarray(Wf1[CEMB:, :].astype(NP_BF16)),
        wf28=np.ascontiguousarray(wf28.reshape(P, 2 * 208)),
        wf38=np.ascontiguousarray(wf38.reshape(P, 2 * 32)),
    )

    in_maps = []
    for c in range(NCORES):
        ic = idx[c * BS:(c + 1) * BS]
        tc_ = tidx[c * BS:(c + 1) * BS]
        m = dict(shared)
        m["u_idx"] = _wrap16((ic & (SLAB - 1)).astype(np.uint16))
        m["u_tidx"] = _wrap16((tc_ & (SLAB - 1)).astype(np.uint16))
        m["qrow_cs"] = np.ascontiguousarray((ic >> 14).astype(NP_BF16))
        m["qrow_dm"] = np.ascontiguousarray((tc_ >> 14).astype(NP_BF16))
        in_maps.append(m)
    return in_maps


def kernel(**inputs):
    nc = _get_nc()
    in_maps = make_in_maps(inputs)
    last_err = None
    for _attempt in range(3):
        try:
            res = run_bass_kernel_spmd(nc, in_maps, core_ids=list(range(NCORES)))
            return np.ascontiguousarray(np.concatenate(
                [res.results[c]["mu_s"].T for c in range(NCORES)], axis=0))
        except Exception as e:  # wedged device sometimes recovers on retry
            last_err = e
    raise last_err
